# revision 1
# baseline (speedup 1.0000x reference)
"""Trainium2 Bass kernel: per-cluster PCA geometry features (segment reduce).

Problem: data [4194304, 6] f32, clusts [32768, 128] int — per cluster of 128
voxels compute: center (mean of xyz), normalized covariance B = A/lmax,
principal axis v0 scaled by dirwt = 1 - lmid/lmax with a sign fix, size.

Strategy: shard the 32768 clusters across 8 NeuronCores (4096 each). Host
pre-partitions voxel coordinates per cluster (a pure permutation). On device:
  phase 1: per-cluster sums/second moments via free-dim reduces (DVE) with
           product planes on ACT (squares) / GPSIMD (cross products).
  eigen:   batched analytic 3x3 symmetric eigensolve on [128, 32] tiles
           (trig method: Arctan+Sin on ACT), eigenvector via cross products.
  phase 2: second pass over voxel planes for the sign criterion
           sc = sum_s x0 * ||xc - x0 v0||.
Cluster c <-> (partition p = c // 32, segment j = c % 32); voxel planes live
as [128, 32, 128] SBUF tiles, per-cluster scalars as [128, 32] tiles
broadcast into plane ops via stride-0 access patterns.
"""
import numpy as np
from contextlib import ExitStack

import concourse.bass as bass
import concourse.bacc as bacc
import concourse.tile as tile
from concourse import mybir
from concourse.bass_utils import run_bass_kernel_spmd

N_CLUSTS = 32768
CLUST_SIZE = 128
N_CORES = 8
C_LOC = N_CLUSTS // N_CORES   # 4096 clusters per core
P = 128                       # SBUF partitions
NSEG = C_LOC // P             # 32 clusters (segments) per partition
V = CLUST_SIZE                # 128 voxels per cluster

F32 = mybir.dt.float32
AF = mybir.ActivationFunctionType
OP = mybir.AluOpType
AX = mybir.AxisListType

TWO_PI_3 = 2.0943951023931953   # 2*pi/3
PI_3 = 1.0471975511965976       # pi/3

_CACHED = {}


def _bcast(t):
    """[P, NSEG] per-cluster tile -> [P, NSEG, V] stride-0 broadcast AP."""
    return t[:, :, None].broadcast_to([P, NSEG, V])


def build_nc():
    nc = bacc.Bacc()
    x_d = nc.dram_tensor("x", [C_LOC, V], F32, kind="ExternalInput").ap()
    y_d = nc.dram_tensor("y", [C_LOC, V], F32, kind="ExternalInput").ap()
    z_d = nc.dram_tensor("z", [C_LOC, V], F32, kind="ExternalInput").ap()
    # voxel-major copies: phase-1 segment sums run on the PE (lhsT = plane
    # chunk, rhs = ones), which contracts over partitions = voxel slots
    xt_d = nc.dram_tensor("xt", [V, C_LOC], F32, kind="ExternalInput").ap()
    yt_d = nc.dram_tensor("yt", [V, C_LOC], F32, kind="ExternalInput").ap()
    zt_d = nc.dram_tensor("zt", [V, C_LOC], F32, kind="ExternalInput").ap()
    feats_d = nc.dram_tensor("feats", [C_LOC, 16], F32, kind="ExternalOutput").ap()

    with tile.TileContext(nc) as tc, ExitStack() as ctx:
        pool = ctx.enter_context(tc.tile_pool(name="main", bufs=1))
        # recycled full-plane scratch (16KB/partition each)
        sp = ctx.enter_context(tc.tile_pool(name="scratch", bufs=4))
        pp = ctx.enter_context(tc.tile_pool(name="psum", bufs=1, space="PSUM"))

        def plane(name):
            return sp.tile([P, NSEG, V], F32, tag="plane", name=name)

        def small(name, pool_=None):
            return (pool_ or pool).tile([P, NSEG], F32, tag=f"s_{name}", name=name)

        # bias constants for Sin activations (activation bias must be an SBUF AP)
        bias_pi2 = pool.tile([P, 1], F32, tag="bias_pi2")
        bias_pi6 = pool.tile([P, 1], F32, tag="bias_pi6")
        nc.gpsimd.memset(bias_pi2[:], 1.5707963267948966)
        nc.gpsimd.memset(bias_pi6[:], 0.5235987755982988)

        Xt = pool.tile([P, C_LOC], F32, tag="Xt")
        Yt = pool.tile([P, C_LOC], F32, tag="Yt")
        Zt = pool.tile([P, C_LOC], F32, tag="Zt")
        nc.sync.dma_start(Xt[:], xt_d)
        nc.sync.dma_start(Yt[:], yt_d)
        nc.sync.dma_start(Zt[:], zt_d)

        X = pool.tile([P, NSEG, V], F32, tag="X")
        Y = pool.tile([P, NSEG, V], F32, tag="Y")
        Z = pool.tile([P, NSEG, V], F32, tag="Z")
        nc.sync.dma_start(X[:], x_d.rearrange("(p s) v -> p s v", p=P))
        nc.sync.dma_start(Y[:], y_d.rearrange("(p s) v -> p s v", p=P))
        nc.sync.dma_start(Z[:], z_d.rearrange("(p s) v -> p s v", p=P))

        ones = pool.tile([P, 1], F32, tag="ones")
        nc.gpsimd.memset(ones[:], 1.0)

        # ---------------- phase 1: sums + second moments on the PE ----------
        # psum[:, k*NSEG + j] = sum over voxels of plane k, cluster chunk j
        psum = pp.tile([P, 9 * NSEG], F32, tag="psums")

        def pe_colsums(plane_t, k):
            for j in range(NSEG):
                nc.tensor.matmul(
                    out=psum[:, k * NSEG + j : k * NSEG + j + 1],
                    lhsT=plane_t[:, j * P : (j + 1) * P],
                    rhs=ones[:, 0:1],
                    start=True,
                    stop=True,
                )

        pe_colsums(Xt, 0)
        pe_colsums(Yt, 1)
        pe_colsums(Zt, 2)
        sqt = sp.tile([P, C_LOC], F32, tag="plane", name="sqt")
        nc.scalar.activation(sqt[:], Xt[:], AF.Square)
        pe_colsums(sqt, 3)
        sqt2 = sp.tile([P, C_LOC], F32, tag="plane", name="sqt2")
        nc.scalar.activation(sqt2[:], Yt[:], AF.Square)
        pe_colsums(sqt2, 4)
        sqt3 = sp.tile([P, C_LOC], F32, tag="plane", name="sqt3")
        nc.scalar.activation(sqt3[:], Zt[:], AF.Square)
        pe_colsums(sqt3, 5)
        prt = sp.tile([P, C_LOC], F32, tag="plane", name="prt")
        nc.gpsimd.tensor_tensor(prt[:], Xt[:], Yt[:], OP.mult)
        pe_colsums(prt, 6)
        prt2 = sp.tile([P, C_LOC], F32, tag="plane", name="prt2")
        nc.gpsimd.tensor_tensor(prt2[:], Xt[:], Zt[:], OP.mult)
        pe_colsums(prt2, 7)
        prt3 = sp.tile([P, C_LOC], F32, tag="plane", name="prt3")
        nc.gpsimd.tensor_tensor(prt3[:], Yt[:], Zt[:], OP.mult)
        pe_colsums(prt3, 8)

        moments = pool.tile([P, 9, NSEG], F32, tag="moments")
        nc.vector.tensor_copy(moments[:], psum[:].rearrange("p (k s) -> p k s", k=9))
        Sx = moments[:, 0]; Sy = moments[:, 1]; Sz = moments[:, 2]
        Mxx = moments[:, 3]; Myy = moments[:, 4]; Mzz = moments[:, 5]
        Mxy = moments[:, 6]; Mxz = moments[:, 7]; Myz = moments[:, 8]

        # ---------------- phase 1.5: A matrix, eigensolve ----------------
        # helpers for tiny [P, NSEG] ops
        def tt(eng, out, a, b, op):
            eng.tensor_tensor(out[:], a[:], b[:], op)

        def act(out, in_, func, bias=0.0, scale=1.0):
            nc.scalar.activation(out[:], in_[:], func, bias=bias, scale=scale)

        inv_s = 1.0 / V
        cx = small("cx"); cy = small("cy"); cz = small("cz")
        nc.vector.tensor_scalar(out=cx[:], in0=Sx[:], scalar1=inv_s, scalar2=None, op0=OP.mult)
        nc.vector.tensor_scalar(out=cy[:], in0=Sy[:], scalar1=inv_s, scalar2=None, op0=OP.mult)
        nc.vector.tensor_scalar(out=cz[:], in0=Sz[:], scalar1=inv_s, scalar2=None, op0=OP.mult)

        # centered second moments: a_ij = M_ij - S_i * S_j / V
        axx = small("axx"); ayy = small("ayy"); azz = small("azz")
        axy = small("axy"); axz = small("axz"); ayz = small("ayz")
        t0 = small("t0"); t1 = small("t1"); t2 = small("t2"); t3 = small("t3")
        act(t0, Sx, AF.Square)
        nc.vector.scalar_tensor_tensor(out=axx[:], in0=t0[:], scalar=-inv_s, in1=Mxx[:], op0=OP.mult, op1=OP.add)
        act(t1, Sy, AF.Square)
        nc.vector.scalar_tensor_tensor(out=ayy[:], in0=t1[:], scalar=-inv_s, in1=Myy[:], op0=OP.mult, op1=OP.add)
        act(t2, Sz, AF.Square)
        nc.vector.scalar_tensor_tensor(out=azz[:], in0=t2[:], scalar=-inv_s, in1=Mzz[:], op0=OP.mult, op1=OP.add)
        tt(nc.gpsimd, t0, Sx, Sy, OP.mult)
        nc.vector.scalar_tensor_tensor(out=axy[:], in0=t0[:], scalar=-inv_s, in1=Mxy[:], op0=OP.mult, op1=OP.add)
        tt(nc.gpsimd, t1, Sx, Sz, OP.mult)
        nc.vector.scalar_tensor_tensor(out=axz[:], in0=t1[:], scalar=-inv_s, in1=Mxz[:], op0=OP.mult, op1=OP.add)
        tt(nc.gpsimd, t2, Sy, Sz, OP.mult)
        nc.vector.scalar_tensor_tensor(out=ayz[:], in0=t2[:], scalar=-inv_s, in1=Myz[:], op0=OP.mult, op1=OP.add)

        # q = tr(A)/3 ; b_ii = a_ii - q
        q = small("q")
        tt(nc.vector, t0, axx, ayy, OP.add)
        tt(nc.vector, t0, t0, azz, OP.add)
        nc.vector.tensor_scalar(out=q[:], in0=t0[:], scalar1=1.0 / 3.0, scalar2=None, op0=OP.mult)
        b11 = small("b11"); b22 = small("b22"); b33 = small("b33")
        tt(nc.vector, b11, axx, q, OP.subtract)
        tt(nc.vector, b22, ayy, q, OP.subtract)
        tt(nc.vector, b33, azz, q, OP.subtract)

        # p2 = b11^2+b22^2+b33^2 + 2*(axy^2+axz^2+ayz^2); p = sqrt(p2/6)
        p2 = small("p2")
        act(t0, b11, AF.Square)
        act(t1, b22, AF.Square)
        act(t2, b33, AF.Square)
        tt(nc.vector, t0, t0, t1, OP.add)
        tt(nc.vector, t0, t0, t2, OP.add)
        act(t1, axy, AF.Square)
        act(t2, axz, AF.Square)
        act(t3, ayz, AF.Square)
        tt(nc.gpsimd, t1, t1, t2, OP.add)
        tt(nc.gpsimd, t1, t1, t3, OP.add)
        nc.vector.scalar_tensor_tensor(out=p2[:], in0=t1[:], scalar=2.0, in1=t0[:], op0=OP.mult, op1=OP.add)
        p = small("p")
        act(p, p2, AF.Sqrt, scale=1.0 / 6.0)
        # newton-refine p (ACT sqrt table has a loose precision budget):
        # p <- 0.5 * (p + (p2/6) / p)
        invp0 = small("invp0")
        nc.vector.reciprocal(invp0[:], p[:])
        nc.vector.tensor_scalar(out=t0[:], in0=p2[:], scalar1=1.0 / 6.0, scalar2=None, op0=OP.mult)
        tt(nc.vector, t0, t0, invp0, OP.mult)
        tt(nc.vector, t0, t0, p, OP.add)
        nc.vector.tensor_scalar(out=p[:], in0=t0[:], scalar1=0.5, scalar2=None, op0=OP.mult)

        invp = small("invp")
        nc.vector.reciprocal(invp[:], p[:])

        # normalized traceless C = (A - qI)/p ; r = det(C)/2 clamped to [-1,1]
        c11 = small("c11"); c22 = small("c22"); c33 = small("c33")
        c12 = small("c12"); c13 = small("c13"); c23 = small("c23")
        tt(nc.vector, c11, b11, invp, OP.mult)
        tt(nc.vector, c22, b22, invp, OP.mult)
        tt(nc.vector, c33, b33, invp, OP.mult)
        tt(nc.gpsimd, c12, axy, invp, OP.mult)
        tt(nc.gpsimd, c13, axz, invp, OP.mult)
        tt(nc.gpsimd, c23, ayz, invp, OP.mult)

        r = small("r")
        tt(nc.vector, t0, c22, c33, OP.mult)
        act(t1, c23, AF.Square)
        tt(nc.vector, t0, t0, t1, OP.subtract)      # m1
        tt(nc.vector, t0, t0, c11, OP.mult)         # c11*m1
        tt(nc.gpsimd, t1, c12, c33, OP.mult)
        tt(nc.gpsimd, t2, c23, c13, OP.mult)
        tt(nc.gpsimd, t1, t1, t2, OP.subtract)      # m2
        tt(nc.gpsimd, t1, t1, c12, OP.mult)         # c12*m2
        tt(nc.vector, t2, c12, c23, OP.mult)
        tt(nc.vector, t3, c22, c13, OP.mult)
        tt(nc.vector, t2, t2, t3, OP.subtract)      # m3
        tt(nc.vector, t2, t2, c13, OP.mult)         # c13*m3
        tt(nc.vector, t0, t0, t1, OP.subtract)
        tt(nc.vector, t0, t0, t2, OP.add)           # det
        nc.vector.tensor_scalar(out=r[:], in0=t0[:], scalar1=0.5, scalar2=1.0, op0=OP.mult, op1=OP.min)
        nc.vector.tensor_scalar(out=r[:], in0=r[:], scalar1=-1.0, scalar2=None, op0=OP.max)

        # theta = acos(r) = 4*arctan( sqrt((1-r)/2) / (1 + sqrt((1+r)/2)) )
        # (quarter-angle form keeps the arctan argument in [0, 1] — the ACT
        # arctan table only covers [-pi/2, pi/2])
        at4 = small("at4")
        nc.vector.tensor_scalar(out=t0[:], in0=r[:], scalar1=-0.5, scalar2=0.5, op0=OP.mult, op1=OP.add)
        nc.vector.tensor_scalar(out=t1[:], in0=r[:], scalar1=0.5, scalar2=0.5, op0=OP.mult, op1=OP.add)
        sa = small("sa"); sb = small("sb")
        act(sa, t0, AF.Sqrt)
        act(sb, t1, AF.Sqrt)
        # newton-refine both sqrts (guarded): s <- 0.5*(s + v/s)
        nc.vector.tensor_scalar(out=sa[:], in0=sa[:], scalar1=1e-30, scalar2=None, op0=OP.max)
        nc.vector.reciprocal(t2[:], sa[:])
        tt(nc.vector, t3, t0, t2, OP.mult)
        tt(nc.vector, sa, sa, t3, OP.add)
        nc.vector.tensor_scalar(out=sa[:], in0=sa[:], scalar1=0.5, scalar2=None, op0=OP.mult)
        nc.vector.tensor_scalar(out=sb[:], in0=sb[:], scalar1=1e-30, scalar2=None, op0=OP.max)
        nc.vector.reciprocal(t2[:], sb[:])
        tt(nc.vector, t3, t1, t2, OP.mult)
        tt(nc.vector, sb, sb, t3, OP.add)
        nc.vector.tensor_scalar(out=sb[:], in0=sb[:], scalar1=0.5, scalar2=1.0, op0=OP.mult, op1=OP.add)  # 1 + sqrt((1+r)/2)
        nc.vector.reciprocal(t2[:], sb[:])
        tt(nc.vector, t3, sa, t2, OP.mult)          # tan(theta/4) in [0, 1]
        act(at4, t3, AF.Arctan)

        # cos(theta/3) = sin(pi/2 - (4/3)*at4);  sin(pi/6 + theta/3) = sin(pi/6 + (4/3)*at4)
        cmax = small("cmax"); smin = small("smin")
        act(cmax, at4, AF.Sin, bias=bias_pi2[:, 0:1], scale=-4.0 / 3.0)
        act(smin, at4, AF.Sin, bias=bias_pi6[:, 0:1], scale=4.0 / 3.0)

        # eigenvalues: w3 = q + 2p*cmax (max), w1 = q - 2p*smin (min), w2 = 3q - w3 - w1
        w3 = small("w3"); w2 = small("w2")
        tt(nc.vector, t0, p, cmax, OP.mult)
        tt(nc.vector, t0, t0, t0, OP.add)  # 2*p*cmax
        tt(nc.vector, w3, q, t0, OP.add)
        tt(nc.gpsimd, t1, p, smin, OP.mult)
        tt(nc.gpsimd, t1, t1, t1, OP.add)
        tt(nc.gpsimd, t1, q, t1, OP.subtract)       # w1
        nc.vector.tensor_scalar(out=t2[:], in0=q[:], scalar1=3.0, scalar2=None, op0=OP.mult)
        tt(nc.vector, t2, t2, w3, OP.subtract)
        tt(nc.vector, w2, t2, t1, OP.subtract)

        invw3 = small("invw3")
        nc.vector.reciprocal(invw3[:], w3[:])
        dirwt = small("dirwt")
        tt(nc.vector, t0, w2, invw3, OP.mult)
        nc.vector.tensor_scalar(out=dirwt[:], in0=t0[:], scalar1=-1.0, scalar2=1.0, op0=OP.mult, op1=OP.add)

        # ---- eigenvector for w3: cross products of rows of (A - w3 I) ----
        d1 = small("d1"); d2 = small("d2"); d3 = small("d3")
        tt(nc.vector, d1, axx, w3, OP.subtract)
        tt(nc.vector, d2, ayy, w3, OP.subtract)
        tt(nc.vector, d3, azz, w3, OP.subtract)

        u1 = small("u1"); u2 = small("u2"); u3 = small("u3")
        tt(nc.vector, u1, axy, ayz, OP.mult)
        tt(nc.gpsimd, t0, d2, axz, OP.mult)
        tt(nc.vector, u1, u1, t0, OP.subtract)
        tt(nc.vector, u2, axy, axz, OP.mult)
        tt(nc.gpsimd, t1, d1, ayz, OP.mult)
        tt(nc.vector, u2, u2, t1, OP.subtract)
        tt(nc.vector, u3, d1, d2, OP.mult)
        act(t2, axy, AF.Square)
        tt(nc.vector, u3, u3, t2, OP.subtract)

        v1 = small("v1"); v2 = small("v2"); v3_ = small("v3_")
        tt(nc.vector, v1, axy, d3, OP.mult)
        tt(nc.gpsimd, t0, axz, ayz, OP.mult)
        tt(nc.vector, v1, v1, t0, OP.subtract)
        act(v2, axz, AF.Square)
        tt(nc.gpsimd, t1, d1, d3, OP.mult)
        tt(nc.vector, v2, v2, t1, OP.subtract)
        tt(nc.vector, v3_, d1, ayz, OP.mult)
        tt(nc.gpsimd, t2, axy, axz, OP.mult)
        tt(nc.vector, v3_, v3_, t2, OP.subtract)

        k1 = small("k1"); k2 = small("k2"); k3 = small("k3")
        tt(nc.vector, k1, d2, d3, OP.mult)
        act(t0, ayz, AF.Square)
        tt(nc.vector, k1, k1, t0, OP.subtract)
        tt(nc.vector, k2, ayz, axz, OP.mult)
        tt(nc.gpsimd, t1, axy, d3, OP.mult)
        tt(nc.vector, k2, k2, t1, OP.subtract)
        tt(nc.vector, k3, axy, ayz, OP.mult)
        tt(nc.gpsimd, t2, d2, axz, OP.mult)
        tt(nc.vector, k3, k3, t2, OP.subtract)

        # squared norms
        nu = small("nu"); nv = small("nv"); nk = small("nk")
        for (n_, e1, e2, e3) in ((nu, u1, u2, u3), (nv, v1, v2, v3_), (nk, k1, k2, k3)):
            act(t0, e1, AF.Square)
            act(t1, e2, AF.Square)
            act(t2, e3, AF.Square)
            tt(nc.vector, t0, t0, t1, OP.add)
            tt(nc.vector, n_, t0, t2, OP.add)

        # pick the largest-norm candidate (select mask must be integer dtype)
        m = pool.tile([P, NSEG], mybir.dt.uint8, tag="s_mask", name="m")
        e1 = small("e1"); e2 = small("e2"); e3 = small("e3"); ne = small("ne")
        tt(nc.vector, m, nv, nu, OP.is_gt)
        nc.vector.select(e1[:], m[:], v1[:], u1[:])
        nc.vector.select(e2[:], m[:], v2[:], u2[:])
        nc.vector.select(e3[:], m[:], v3_[:], u3[:])
        nc.vector.select(ne[:], m[:], nv[:], nu[:])
        tt(nc.vector, m, nk, ne, OP.is_gt)
        nc.vector.select(e1[:], m[:], k1[:], e1[:])
        nc.vector.select(e2[:], m[:], k2[:], e2[:])
        nc.vector.select(e3[:], m[:], k3[:], e3[:])
        nc.vector.select(ne[:], m[:], nk[:], ne[:])

        # normalize: v0 = e / sqrt(ne)   (sqrt + newton refine)
        act(t0, ne, AF.Sqrt)
        nc.vector.tensor_scalar(out=t0[:], in0=t0[:], scalar1=1e-30, scalar2=None, op0=OP.max)
        nc.vector.reciprocal(t1[:], t0[:])
        tt(nc.vector, t2, ne, t1, OP.mult)
        tt(nc.vector, t0, t0, t2, OP.add)
        nc.vector.tensor_scalar(out=t0[:], in0=t0[:], scalar1=0.5, scalar2=1e-30, op0=OP.mult, op1=OP.max)
        invn = small("invn")
        nc.vector.reciprocal(invn[:], t0[:])
        v0x = small("v0x"); v0y = small("v0y"); v0z = small("v0z")
        tt(nc.vector, v0x, e1, invn, OP.mult)
        tt(nc.vector, v0y, e2, invn, OP.mult)
        tt(nc.vector, v0z, e3, invn, OP.mult)

        # ---------------- phase 2: sign criterion ----------------
        # center in place: X <- X - cx (broadcast)
        nc.gpsimd.tensor_tensor(X[:], X[:], _bcast(cx), OP.subtract)
        nc.gpsimd.tensor_tensor(Y[:], Y[:], _bcast(cy), OP.subtract)
        nc.gpsimd.tensor_tensor(Z[:], Z[:], _bcast(cz), OP.subtract)

        # x0 = Xc*v0x + Yc*v0y + Zc*v0z
        x0 = plane("x0")
        w0 = plane("w0")
        nc.vector.tensor_tensor(x0[:], X[:], _bcast(v0x), OP.mult)
        nc.vector.tensor_tensor(w0[:], Y[:], _bcast(v0y), OP.mult)
        nc.vector.tensor_tensor(x0[:], x0[:], w0[:], OP.add)
        nc.vector.tensor_tensor(w0[:], Z[:], _bcast(v0z), OP.mult)
        nc.vector.tensor_tensor(x0[:], x0[:], w0[:], OP.add)

        # n2 = (Xc^2+Yc^2+Zc^2) - x0^2, clamped at 0; np0 = sqrt(n2)
        r2 = plane("r2")
        s1 = plane("s1")
        s2 = plane("s2")
        nc.scalar.activation(r2[:], X[:], AF.Square)
        nc.scalar.activation(s1[:], Y[:], AF.Square)
        nc.scalar.activation(s2[:], Z[:], AF.Square)
        nc.vector.tensor_tensor(r2[:], r2[:], s1[:], OP.add)
        nc.gpsimd.tensor_tensor(r2[:], r2[:], s2[:], OP.add)
        nc.scalar.activation(s1[:], x0[:], AF.Square)
        nc.vector.tensor_tensor(r2[:], r2[:], s1[:], OP.subtract)
        nc.vector.tensor_scalar(out=r2[:], in0=r2[:], scalar1=0.0, scalar2=None, op0=OP.max)
        np0 = plane("np0")
        nc.scalar.activation(np0[:], r2[:], AF.Sqrt)
        # pr = x0 * np0 ; sc = sum_s pr
        nc.vector.tensor_tensor(np0[:], np0[:], x0[:], OP.mult)
        sc = small("sc")
        nc.vector.tensor_reduce(sc[:], np0[:], axis=AX.X, op=OP.add)

        # ---------------- finalize: feats [P, NSEG, 16] ----------------
        feats = pool.tile([P, NSEG, 16], F32, tag="feats")
        # fac = dirwt * (sc < 0 ? -1 : 1)
        fac = small("fac")
        nc.vector.tensor_scalar(out=t0[:], in0=sc[:], scalar1=0.0, scalar2=-2.0, op0=OP.is_lt, op1=OP.mult)
        nc.vector.tensor_scalar(out=t0[:], in0=t0[:], scalar1=1.0, scalar2=None, op0=OP.add)
        tt(nc.vector, fac, t0, dirwt, OP.mult)

        nc.vector.tensor_copy(feats[:, :, 0], cx[:])
        nc.vector.tensor_copy(feats[:, :, 1], cy[:])
        nc.vector.tensor_copy(feats[:, :, 2], cz[:])
        # B = A / w3  (9 entries, B is symmetric)
        tt(nc.vector, t0, axx, invw3, OP.mult)
        nc.vector.tensor_copy(feats[:, :, 3], t0[:])
        tt(nc.vector, t0, axy, invw3, OP.mult)
        nc.vector.tensor_copy(feats[:, :, 4], t0[:])
        nc.vector.tensor_copy(feats[:, :, 6], t0[:])
        tt(nc.vector, t0, axz, invw3, OP.mult)
        nc.vector.tensor_copy(feats[:, :, 5], t0[:])
        nc.vector.tensor_copy(feats[:, :, 9], t0[:])
        tt(nc.vector, t0, ayy, invw3, OP.mult)
        nc.vector.tensor_copy(feats[:, :, 7], t0[:])
        tt(nc.vector, t0, ayz, invw3, OP.mult)
        nc.vector.tensor_copy(feats[:, :, 8], t0[:])
        nc.vector.tensor_copy(feats[:, :, 10], t0[:])
        tt(nc.vector, t0, azz, invw3, OP.mult)
        nc.vector.tensor_copy(feats[:, :, 11], t0[:])
        tt(nc.vector, t0, v0x, fac, OP.mult)
        nc.vector.tensor_copy(feats[:, :, 12], t0[:])
        tt(nc.vector, t0, v0y, fac, OP.mult)
        nc.vector.tensor_copy(feats[:, :, 13], t0[:])
        tt(nc.vector, t0, v0z, fac, OP.mult)
        nc.vector.tensor_copy(feats[:, :, 14], t0[:])
        size_t = small("size_t")
        nc.gpsimd.memset(size_t[:], float(V))
        nc.vector.tensor_copy(feats[:, :, 15], size_t[:])

        nc.sync.dma_start(feats_d.rearrange("(p s) k -> p s k", p=P), feats[:])

    if not nc.is_finalized():
        nc.finalize()
    return nc


def kernel(data: np.ndarray, clusts: np.ndarray) -> np.ndarray:
    data = np.ascontiguousarray(np.asarray(data, dtype=np.float32))
    clusts_np = np.asarray(clusts)
    C, S = clusts_np.shape
    assert (C, S) == (N_CLUSTS, CLUST_SIZE), (C, S)

    # host-side pre-partition: gather each cluster's voxel coordinates
    vox = data[:, 1:4]
    g = vox[clusts_np.reshape(-1).astype(np.int64)].reshape(C, S, 3)
    xs = np.ascontiguousarray(g[:, :, 0])
    ys = np.ascontiguousarray(g[:, :, 1])
    zs = np.ascontiguousarray(g[:, :, 2])

    if "nc" not in _CACHED:
        _CACHED["nc"] = build_nc()
    nc = _CACHED["nc"]

    def tmajor(a):
        # voxel-major plane whose column j*128+m is cluster m*32+j, so the
        # PE column-sum (chunk j -> psum partition m) lands exactly at the
        # kernel's cluster slot (partition m, segment j)
        return np.ascontiguousarray(
            a.reshape(P, NSEG, V).transpose(2, 1, 0).reshape(V, C_LOC))

    in_maps = []
    for c in range(N_CORES):
        sl = slice(c * C_LOC, (c + 1) * C_LOC)
        in_maps.append({
            "x": xs[sl], "y": ys[sl], "z": zs[sl],
            "xt": tmajor(xs[sl]),
            "yt": tmajor(ys[sl]),
            "zt": tmajor(zs[sl]),
        })

    res = run_bass_kernel_spmd(nc, in_maps, list(range(N_CORES)))
    out = np.concatenate([res.results[c]["feats"] for c in range(N_CORES)], axis=0)
    return out.astype(np.float32)



# revision 10
# speedup vs baseline: 1.6020x; 1.6020x over previous
"""Trainium2 Bass kernel: per-cluster PCA geometry features (segment reduce).

Problem: data [4194304, 6] f32, clusts [32768, 128] int — per cluster of 128
voxels compute: center (mean of xyz), normalized covariance B = A/lmax,
principal axis v0 scaled by dirwt = 1 - lmid/lmax with a sign fix, size.

Strategy (v4): shard the 32768 clusters across 8 NeuronCores (4096 each).
Host pre-gathers each cluster's voxel coords (pure permutation), casts to
bf16, and ships TWO layouts per core:
  voxel-major  xt/yt/zt [128 vox, 4096 clusters] — phase-1 moment sums run
    on the PE (column sums via ones-rhs matmuls, nearly free).
  cluster-major xc/yc/zc [128 part, 128 vox, 16 seg] per half — phase-2
    element ops. Segment-INNERMOST layout keeps every DVE operand's last AP
    dim stride-1 so bf16 ops hit the 2x DVE mode, including per-cluster
    broadcasts (stride-0 on the middle/voxel dim only).
Cluster c = g*128 + q maps to (partition q, segment g), matching the PE
column-sum output layout, so moments land directly where the eigensolve
([128, 32] fp32 small-tile analytic 3x3 solve, trig method) wants them.
Input DMAs are split across the SP/ACT/Pool issue queues so transfers
overlap; work is split across DVE/ACT/Pool by measured cost-model rates
(Pool subtract is cheaper than mult); ACT table switches (sqrt<->trig
sets) are batched; feature values are written straight into the output
tile; tails and output DMA run per half.
"""
import numpy as np
from contextlib import ExitStack

import concourse.bass as bass
import concourse.bacc as bacc
import concourse.tile as tile
from concourse import mybir
from concourse.bass_utils import run_bass_kernel_spmd

N_CLUSTS = 32768
CLUST_SIZE = 128
N_CORES = 8
C_LOC = N_CLUSTS // N_CORES   # 4096 clusters per core
P = 128                       # SBUF partitions
NSEG = C_LOC // P             # 32 clusters (segments) per partition
V = CLUST_SIZE                # 128 voxels per cluster
NH = 2                        # halves for pipelining
GH = NSEG // NH               # 16 segments per half
CH = C_LOC // NH              # 2048 clusters per half

F32 = mybir.dt.float32
BF16 = mybir.dt.bfloat16
U8 = mybir.dt.uint8
AF = mybir.ActivationFunctionType
OP = mybir.AluOpType
AX = mybir.AxisListType

PI_2 = 1.5707963267948966
PI_6 = 0.5235987755982988

_CACHED = {}


def build_nc():
    nc = bacc.Bacc()
    xt_d = nc.dram_tensor("xt", [V, C_LOC], BF16, kind="ExternalInput").ap()
    yt_d = nc.dram_tensor("yt", [V, C_LOC], BF16, kind="ExternalInput").ap()
    zt_d = nc.dram_tensor("zt", [V, C_LOC], BF16, kind="ExternalInput").ap()
    xc_d = nc.dram_tensor("xc", [NH, P, V, GH], BF16, kind="ExternalInput").ap()
    yc_d = nc.dram_tensor("yc", [NH, P, V, GH], BF16, kind="ExternalInput").ap()
    zc_d = nc.dram_tensor("zc", [NH, P, V, GH], BF16, kind="ExternalInput").ap()
    feats_d = nc.dram_tensor("feats", [NSEG, P, 16], F32, kind="ExternalOutput").ap()

    with tile.TileContext(nc) as tc, ExitStack() as ctx:
        pool = ctx.enter_context(tc.tile_pool(name="main", bufs=1))
        sp = ctx.enter_context(tc.tile_pool(name="p1s", bufs=6))
        p2p = ctx.enter_context(tc.tile_pool(name="p2s", bufs=1))
        pp = ctx.enter_context(tc.tile_pool(name="psum", bufs=2, space="PSUM"))

        D = nc.vector   # DVE
        A = nc.scalar   # Activation
        G = nc.gpsimd   # Pool

        ones = pool.tile([P, 1], BF16, tag="ones")
        G.memset(ones[:], 1.0)
        bias_pi2 = pool.tile([P, 1], F32, tag="bias_pi2")
        bias_pi6 = pool.tile([P, 1], F32, tag="bias_pi6")
        G.memset(bias_pi2[:], PI_2)
        G.memset(bias_pi6[:], PI_6)

        # ---- input DMAs, split across issue queues so transfers overlap ----
        vm = {}   # (coord, half) -> [P, CH] bf16 voxel-major
        cm = {}   # (coord, half) -> [P, V, GH] bf16 cluster-major seg-inner
        for h in range(NH):
            for k, (name, d) in enumerate(
                    (("x", xt_d), ("y", yt_d), ("z", zt_d))):
                t = pool.tile([P, CH], BF16, tag=f"vm_{name}{h}", name=f"vm_{name}{h}")
                nc.sync.dma_start(t[:], d[:, h * CH:(h + 1) * CH])
                vm[(k, h)] = t
        for h in range(NH):
            eng = nc.scalar if h == 0 else nc.gpsimd
            for k, (name, d) in enumerate(
                    (("x", xc_d), ("y", yc_d), ("z", zc_d))):
                t = pool.tile([P, V, GH], BF16, tag=f"cm_{name}{h}", name=f"cm_{name}{h}")
                eng.dma_start(t[:], d[h])
                cm[(k, h)] = t

        # ---- shared tiles / helpers ----
        ps = [pp.tile([P, 9 * GH], F32, tag=f"ps{h}", name=f"ps{h}")
              for h in range(NH)]
        moments = pool.tile([P, 9, NSEG], F32, tag="moments")
        Sx = moments[:, 0]; Sy = moments[:, 1]; Sz = moments[:, 2]
        Mxx = moments[:, 3]; Myy = moments[:, 4]; Mzz = moments[:, 5]
        Mxy = moments[:, 6]; Mxz = moments[:, 7]; Myz = moments[:, 8]

        feats = pool.tile([P, NSEG, 16], F32, tag="feats")

        def small(name, dt=F32):
            return pool.tile([P, NSEG], dt, tag=f"s_{name}", name=name)

        def ap(x):
            return x[:] if hasattr(x, "tag") else x

        def tt(eng, out, a, b, op):
            eng.tensor_tensor(ap(out), ap(a), ap(b), op)

        def ts(eng, out, in0, s1, s2=None, op0=OP.mult, op1=None):
            kw = dict(out=ap(out), in0=ap(in0), scalar1=s1, scalar2=s2, op0=op0)
            if op1 is not None:
                kw["op1"] = op1
            eng.tensor_scalar(**kw)

        def stt(eng, out, in0, s, in1, op0, op1):
            eng.scalar_tensor_tensor(out=ap(out), in0=ap(in0), scalar=s,
                                     in1=ap(in1), op0=op0, op1=op1)

        inv_s = 1.0 / V
        cxb = small("cxb", BF16); cyb = small("cyb", BF16); czb = small("czb", BF16)

        # ---- phase 1: moments via ACT/DVE/Pool products + PE column sums ----
        def colsum(h, plane, k):
            # column sums of [128, CH] plane: group g -> psum[:, k*GH+g]
            for g in range(GH):
                nc.tensor.matmul(
                    out=ps[h][:, k * GH + g: k * GH + g + 1],
                    lhsT=plane[:, g * P:(g + 1) * P],
                    rhs=ones[:, 0:1], start=True, stop=True)

        def p1(h):
            x, y, z = vm[(0, h)], vm[(1, h)], vm[(2, h)]
            colsum(h, x, 0)
            colsum(h, y, 1)
            colsum(h, z, 2)
            sq = sp.tile([P, CH], BF16, tag="p1s", name=f"sqx{h}")
            A.activation(sq[:], x[:], AF.Square)
            colsum(h, sq, 3)
            sq = sp.tile([P, CH], BF16, tag="p1s", name=f"sqy{h}")
            A.activation(sq[:], y[:], AF.Square)
            colsum(h, sq, 4)
            sq = sp.tile([P, CH], BF16, tag="p1s", name=f"sqz{h}")
            D.tensor_tensor(sq[:], z[:], z[:], OP.mult)
            colsum(h, sq, 5)
            pr = sp.tile([P, CH], BF16, tag="p1s", name=f"cxy{h}")
            D.tensor_tensor(pr[:], x[:], y[:], OP.mult)
            colsum(h, pr, 6)
            pr = sp.tile([P, CH], BF16, tag="p1s", name=f"cxz{h}")
            G.tensor_tensor(pr[:], x[:], z[:], OP.mult)
            colsum(h, pr, 7)
            pr = sp.tile([P, CH], BF16, tag="p1s", name=f"cyz{h}")
            G.tensor_tensor(pr[:], y[:], z[:], OP.mult)
            colsum(h, pr, 8)
            D.tensor_copy(
                moments[:, :, h * GH:(h + 1) * GH],
                ps[h][:].rearrange("p (k g) -> p k g", k=9))

        def e_early(h):
            hs = slice(h * GH, (h + 1) * GH)
            ts(D, feats[:, hs, 0], Sx[:, hs], inv_s)
            ts(D, feats[:, hs, 1], Sy[:, hs], inv_s)
            ts(D, feats[:, hs, 2], Sz[:, hs], inv_s)
            D.tensor_copy(cxb[:, hs], feats[:, hs, 0])
            D.tensor_copy(cyb[:, hs], feats[:, hs, 1])
            D.tensor_copy(czb[:, hs], feats[:, hs, 2])

        p1(0)
        e_early(0)
        p1(1)
        e_early(1)

        # ---- phase 2a (needs only the centers from the eigen stage) ----
        def bc(t, h):
            return t[:, None, h * GH:(h + 1) * GH].broadcast_to([P, V, GH])

        Xc = [None] * NH; Yc = [None] * NH; Zc = [None] * NH; Ssum = [None] * NH

        def p2a(h):
            x, y, z = cm[(0, h)], cm[(1, h)], cm[(2, h)]
            Xc[h] = p2p.tile([P, V, GH], BF16, tag=f"Xc{h}", name=f"Xc{h}")
            Yc[h] = p2p.tile([P, V, GH], BF16, tag=f"Yc{h}", name=f"Yc{h}")
            Zc[h] = p2p.tile([P, V, GH], BF16, tag=f"Zc{h}", name=f"Zc{h}")
            G.tensor_tensor(Xc[h][:], x[:], bc(cxb, h), OP.subtract)
            G.tensor_tensor(Yc[h][:], y[:], bc(cyb, h), OP.subtract)
            G.tensor_tensor(Zc[h][:], z[:], bc(czb, h), OP.subtract)
            sx = p2p.tile([P, V, GH], BF16, tag="sx_s", name=f"sx{h}")
            sy = p2p.tile([P, V, GH], BF16, tag="sy_s", name=f"sy{h}")
            sz = p2p.tile([P, V, GH], BF16, tag="sz_s", name=f"sz{h}")
            A.activation(sx[:], Xc[h][:], AF.Square)
            A.activation(sy[:], Yc[h][:], AF.Square)
            D.tensor_tensor(sz[:], Zc[h][:], Zc[h][:], OP.mult)
            s1 = p2p.tile([P, V, GH], BF16, tag="s1_s", name=f"s1{h}")
            D.tensor_tensor(s1[:], sx[:], sy[:], OP.add)
            Ssum[h] = p2p.tile([P, V, GH], BF16, tag=f"s{h}", name=f"s{h}")
            G.tensor_tensor(Ssum[h][:], s1[:], sz[:], OP.add)

        p2a(0)
        p2a(1)

        # ---- eigen stage E-rest: [128, 32] fp32 analytic 3x3 eigensolve ----
        axx = small("axx"); ayy = small("ayy"); azz = small("azz")
        axy = small("axy"); axz = small("axz"); ayz = small("ayz")
        t0 = small("t0"); t1 = small("t1"); t2 = small("t2")
        t3 = small("t3"); t4 = small("t4"); t5 = small("t5")
        tt(G, t0, Sx, Sx, OP.mult)
        stt(D, axx, t0, -inv_s, Mxx, OP.mult, OP.add)
        tt(G, t1, Sy, Sy, OP.mult)
        stt(D, ayy, t1, -inv_s, Myy, OP.mult, OP.add)
        tt(G, t2, Sz, Sz, OP.mult)
        stt(D, azz, t2, -inv_s, Mzz, OP.mult, OP.add)
        tt(G, t3, Sx, Sy, OP.mult)
        stt(D, axy, t3, -inv_s, Mxy, OP.mult, OP.add)
        tt(G, t4, Sx, Sz, OP.mult)
        stt(D, axz, t4, -inv_s, Mxz, OP.mult, OP.add)
        tt(G, t5, Sy, Sz, OP.mult)
        stt(D, ayz, t5, -inv_s, Myz, OP.mult, OP.add)

        q = small("q")
        tt(D, t0, axx, ayy, OP.add)
        tt(D, t0, t0, azz, OP.add)
        ts(D, q, t0, 1.0 / 3.0)
        b11 = small("b11"); b22 = small("b22"); b33 = small("b33")
        tt(D, b11, axx, q, OP.subtract)
        tt(D, b22, ayy, q, OP.subtract)
        tt(D, b33, azz, q, OP.subtract)

        tt(G, t0, b11, b11, OP.mult)
        tt(G, t1, b22, b22, OP.mult)
        tt(G, t2, b33, b33, OP.mult)
        tt(G, t3, axy, axy, OP.mult)
        tt(G, t4, axz, axz, OP.mult)
        tt(G, t5, ayz, ayz, OP.mult)
        tt(D, t0, t0, t1, OP.add)
        tt(D, t0, t0, t2, OP.add)
        tt(G, t3, t3, t4, OP.add)
        tt(G, t3, t3, t5, OP.add)
        p2t = small("p2t")
        stt(D, p2t, t3, 2.0, t0, OP.mult, OP.add)
        p_ = small("p_")
        A.activation(p_[:], p2t[:], AF.Sqrt, scale=1.0 / 6.0)
        invp = small("invp")
        D.reciprocal(invp[:], p_[:])

        c11 = small("c11"); c22 = small("c22"); c33 = small("c33")
        c12 = small("c12"); c13 = small("c13"); c23 = small("c23")
        tt(D, c11, b11, invp, OP.mult)
        tt(D, c22, b22, invp, OP.mult)
        tt(D, c33, b33, invp, OP.mult)
        tt(G, c12, axy, invp, OP.mult)
        tt(G, c13, axz, invp, OP.mult)
        tt(G, c23, ayz, invp, OP.mult)

        # det(C)/2 -> r, clamped to [-1, 1]
        tt(D, t0, c22, c33, OP.mult)
        tt(G, t1, c23, c23, OP.mult)
        tt(D, t0, t0, t1, OP.subtract)
        tt(D, t0, t0, c11, OP.mult)
        tt(G, t2, c12, c33, OP.mult)
        tt(G, t3, c23, c13, OP.mult)
        tt(G, t2, t2, t3, OP.subtract)
        tt(G, t2, t2, c12, OP.mult)
        tt(D, t4, c12, c23, OP.mult)
        tt(D, t5, c22, c13, OP.mult)
        tt(D, t4, t4, t5, OP.subtract)
        tt(D, t4, t4, c13, OP.mult)
        tt(D, t0, t0, t2, OP.subtract)
        tt(D, t0, t0, t4, OP.add)
        r = small("r")
        ts(D, r, t0, 0.5, 1.0, OP.mult, OP.min)
        ts(D, r, r, -1.0, None, OP.max)

        # theta/4 = arctan(sqrt((1-r)/2) / (1 + sqrt((1+r)/2)))
        ts(D, t0, r, -0.5, 0.5, OP.mult, OP.add)
        ts(D, t1, r, 0.5, 0.5, OP.mult, OP.add)
        sa = small("sa"); sb = small("sb")
        A.activation(sa[:], t0[:], AF.Sqrt)
        A.activation(sb[:], t1[:], AF.Sqrt)
        ts(D, sb, sb, 1.0, None, OP.add)
        D.reciprocal(t2[:], sb[:])
        tt(D, t3, sa, t2, OP.mult)
        at4 = small("at4")
        A.activation(at4[:], t3[:], AF.Arctan)
        cmax = small("cmax"); smin = small("smin")
        A.activation(cmax[:], at4[:], AF.Sin, bias=bias_pi2[:, 0:1], scale=-4.0 / 3.0)
        A.activation(smin[:], at4[:], AF.Sin, bias=bias_pi6[:, 0:1], scale=4.0 / 3.0)

        w3 = small("w3"); w2 = small("w2")
        tt(D, t0, p_, cmax, OP.mult)
        stt(D, w3, t0, 2.0, q, OP.mult, OP.add)
        tt(G, t1, p_, smin, OP.mult)
        stt(D, t1, t1, -2.0, q, OP.mult, OP.add)      # w1
        stt(D, t2, q, 3.0, w3, OP.mult, OP.subtract)  # 3q - w3
        tt(D, w2, t2, t1, OP.subtract)
        invw3 = small("invw3")
        D.reciprocal(invw3[:], w3[:])
        dirwt = small("dirwt")
        tt(D, t0, w2, invw3, OP.mult)
        ts(D, dirwt, t0, -1.0, 1.0, OP.mult, OP.add)

        # B = A / w3 (9 entries, symmetric) written straight into feats
        tt(D, feats[:, :, 3], axx, invw3, OP.mult)
        tt(D, feats[:, :, 4], axy, invw3, OP.mult)
        A.copy(feats[:, :, 6], feats[:, :, 4])
        tt(D, feats[:, :, 5], axz, invw3, OP.mult)
        A.copy(feats[:, :, 9], feats[:, :, 5])
        tt(D, feats[:, :, 7], ayy, invw3, OP.mult)
        tt(D, feats[:, :, 8], ayz, invw3, OP.mult)
        A.copy(feats[:, :, 10], feats[:, :, 8])
        tt(D, feats[:, :, 11], azz, invw3, OP.mult)

        # principal eigenvector: cross products of rows of (A - w3 I)
        d1 = small("d1"); d2 = small("d2"); d3 = small("d3")
        tt(D, d1, axx, w3, OP.subtract)
        tt(D, d2, ayy, w3, OP.subtract)
        tt(D, d3, azz, w3, OP.subtract)
        u1 = small("u1"); u2 = small("u2"); u3 = small("u3")
        tt(G, t0, axy, ayz, OP.mult)
        tt(G, t1, d2, axz, OP.mult)
        tt(G, u1, t0, t1, OP.subtract)
        tt(D, t2, axy, axz, OP.mult)
        tt(D, t3, d1, ayz, OP.mult)
        tt(D, u2, t2, t3, OP.subtract)
        tt(G, t4, d1, d2, OP.mult)
        tt(G, t5, axy, axy, OP.mult)
        tt(G, u3, t4, t5, OP.subtract)
        k1 = small("k1"); k2 = small("k2")
        tt(D, t0, d2, d3, OP.mult)
        tt(D, t1, ayz, ayz, OP.mult)
        tt(D, k1, t0, t1, OP.subtract)
        tt(G, t2, ayz, axz, OP.mult)
        tt(G, t3, axy, d3, OP.mult)
        tt(G, k2, t2, t3, OP.subtract)
        # k3 = u1 (same formula)
        nu = small("nu"); nk = small("nk")
        nu1 = small("nu1")
        tt(D, nu1, u1, u1, OP.mult)
        tt(D, t0, u2, u2, OP.mult)
        tt(D, t1, u3, u3, OP.mult)
        tt(D, t0, t0, t1, OP.add)
        tt(D, nu, t0, nu1, OP.add)
        tt(G, t2, k1, k1, OP.mult)
        tt(G, t3, k2, k2, OP.mult)
        tt(G, t2, t2, t3, OP.add)
        tt(G, nk, t2, nu1, OP.add)
        m = small("m", U8)
        tt(D, m, nk, nu, OP.is_gt)
        e1 = small("e1"); e2 = small("e2"); e3 = small("e3"); ne = small("ne")
        D.select(e1[:], m[:], k1[:], u1[:])
        D.select(e2[:], m[:], k2[:], u2[:])
        D.select(e3[:], m[:], u1[:], u3[:])
        D.select(ne[:], m[:], nk[:], nu[:])
        rsn = small("rsn")
        A.activation(rsn[:], ne[:], AF.Sqrt)
        ts(D, rsn, rsn, 1e-30, None, OP.max)
        invn = small("invn")
        D.reciprocal(invn[:], rsn[:])
        v0x = small("v0x"); v0y = small("v0y"); v0z = small("v0z")
        tt(D, v0x, e1, invn, OP.mult)
        tt(D, v0y, e2, invn, OP.mult)
        tt(D, v0z, e3, invn, OP.mult)
        v0xb = small("v0xb", BF16); v0yb = small("v0yb", BF16)
        v0zb = small("v0zb", BF16)
        D.tensor_copy(v0xb[:], v0x[:])
        D.tensor_copy(v0yb[:], v0y[:])
        D.tensor_copy(v0zb[:], v0z[:])

        # ---- phase 2b: projections, residual norms, sign criterion ----
        sc = small("sc")
        G.memset(feats[:, :, 15], float(V))

        def p2b(h):
            a1 = p2p.tile([P, V, GH], BF16, tag="a1_s", name=f"a1{h}")
            a2 = p2p.tile([P, V, GH], BF16, tag="a2_s", name=f"a2{h}")
            a3 = p2p.tile([P, V, GH], BF16, tag="a3_s", name=f"a3{h}")
            D.tensor_tensor(a1[:], Xc[h][:], bc(v0xb, h), OP.mult)
            G.tensor_tensor(a2[:], Yc[h][:], bc(v0yb, h), OP.mult)
            D.tensor_tensor(a3[:], Zc[h][:], bc(v0zb, h), OP.mult)
            x0 = p2p.tile([P, V, GH], BF16, tag="x0_s", name=f"x0{h}")
            D.tensor_tensor(x0[:], a1[:], a2[:], OP.add)
            D.tensor_tensor(x0[:], x0[:], a3[:], OP.add)
            q2 = p2p.tile([P, V, GH], BF16, tag="q2_s", name=f"q2{h}")
            A.activation(q2[:], x0[:], AF.Square)
            n2 = p2p.tile([P, V, GH], BF16, tag="n2_s", name=f"n2{h}")
            G.tensor_tensor(n2[:], Ssum[h][:], q2[:], OP.subtract)
            ts(D, n2, n2, 0.0, None, OP.max)
            np0 = p2p.tile([P, V, GH], BF16, tag="np0_s", name=f"np0{h}")
            A.activation(np0[:], n2[:], AF.Sqrt)
            pr = p2p.tile([P, V, GH], BF16, tag="pr_s", name=f"pr{h}")
            D.tensor_tensor(pr[:], x0[:], np0[:], OP.mult)
            D.tensor_reduce(sc[:, h * GH:(h + 1) * GH],
                            pr[:].rearrange("p v g -> p g v"),
                            axis=AX.X, op=OP.add)

        def tail(h):
            hs = slice(h * GH, (h + 1) * GH)
            ts(D, t0[:, hs], sc[:, hs], 0.0, -2.0, OP.is_lt, OP.mult)
            ts(D, t0[:, hs], t0[:, hs], 1.0, None, OP.add)
            tt(D, t1[:, hs], t0[:, hs], dirwt[:, hs], OP.mult)    # fac
            tt(D, feats[:, hs, 12], v0x[:, hs], t1[:, hs], OP.mult)
            tt(D, feats[:, hs, 13], v0y[:, hs], t1[:, hs], OP.mult)
            tt(D, feats[:, hs, 14], v0z[:, hs], t1[:, hs], OP.mult)
            nc.sync.dma_start(
                feats_d[h * GH:(h + 1) * GH].rearrange("g q f -> q g f"),
                feats[:, hs, :])

        p2b(0)
        tail(0)
        p2b(1)
        tail(1)

    if not nc.is_finalized():
        nc.finalize()
    return nc


def kernel(data: np.ndarray, clusts: np.ndarray) -> np.ndarray:
    import ml_dtypes
    data = np.asarray(data, dtype=np.float32)
    clusts_np = np.asarray(clusts)
    C, S = clusts_np.shape
    assert (C, S) == (N_CLUSTS, CLUST_SIZE), (C, S)

    vox = data[:, 1:4]
    g3 = vox[clusts_np.reshape(-1).astype(np.int64)].reshape(C, S, 3)
    g3 = g3.astype(ml_dtypes.bfloat16)

    if "nc" not in _CACHED:
        _CACHED["nc"] = build_nc()
    nc = _CACHED["nc"]

    in_maps = []
    for c in range(N_CORES):
        a = g3[c * C_LOC:(c + 1) * C_LOC]          # [4096, 128, 3]
        vmt = np.ascontiguousarray(a.transpose(1, 0, 2))  # [128 vox, 4096, 3]
        # cluster-major seg-inner: [h, q, v, g] with c = (h*GH+g)*128 + q
        b = a.reshape(NH, GH, P, V, 3).transpose(0, 2, 3, 1, 4)
        b = np.ascontiguousarray(b)                # [2, 128, 128, 16, 3]
        in_maps.append({
            "xt": np.ascontiguousarray(vmt[:, :, 0]),
            "yt": np.ascontiguousarray(vmt[:, :, 1]),
            "zt": np.ascontiguousarray(vmt[:, :, 2]),
            "xc": np.ascontiguousarray(b[..., 0]),
            "yc": np.ascontiguousarray(b[..., 1]),
            "zc": np.ascontiguousarray(b[..., 2]),
        })

    res = run_bass_kernel_spmd(nc, in_maps, list(range(N_CORES)))
    out = np.concatenate(
        [res.results[c]["feats"].reshape(C_LOC, 16) for c in range(N_CORES)],
        axis=0)
    return out.astype(np.float32)


# revision 12
# speedup vs baseline: 1.6716x; 1.0434x over previous
"""Trainium2 Bass kernel: per-cluster PCA geometry features (segment reduce).

Problem: data [4194304, 6] f32, clusts [32768, 128] int — per cluster of 128
voxels compute: center (mean of xyz), normalized covariance B = A/lmax,
principal axis v0 scaled by dirwt = 1 - lmid/lmax with a sign fix, size.

Strategy (v4): shard the 32768 clusters across 8 NeuronCores (4096 each).
Host pre-gathers each cluster's voxel coords (pure permutation), casts to
bf16, and ships TWO layouts per core:
  voxel-major  xt/yt/zt [128 vox, 4096 clusters] — phase-1 moment sums run
    on the PE (column sums via ones-rhs matmuls, nearly free).
  cluster-major xc/yc/zc [128 part, 128 vox, 16 seg] per half — phase-2
    element ops. Segment-INNERMOST layout keeps every DVE operand's last AP
    dim stride-1 so bf16 ops hit the 2x DVE mode, including per-cluster
    broadcasts (stride-0 on the middle/voxel dim only).
Cluster c = g*128 + q maps to (partition q, segment g), matching the PE
column-sum output layout, so moments land directly where the eigensolve
([128, 32] fp32 small-tile analytic 3x3 solve, trig method) wants them.
Input DMAs are split across the SP/ACT/Pool issue queues so transfers
overlap; work is split across DVE/ACT/Pool by measured cost-model rates
(Pool subtract is cheaper than mult); ACT table switches (sqrt<->trig
sets) are batched; feature values are written straight into the output
tile; tails and output DMA run per half.
"""
import numpy as np
from contextlib import ExitStack

import concourse.bass as bass
import concourse.bacc as bacc
import concourse.tile as tile
from concourse import mybir
from concourse.bass_utils import run_bass_kernel_spmd

N_CLUSTS = 32768
CLUST_SIZE = 128
N_CORES = 8
C_LOC = N_CLUSTS // N_CORES   # 4096 clusters per core
P = 128                       # SBUF partitions
NSEG = C_LOC // P             # 32 clusters (segments) per partition
V = CLUST_SIZE                # 128 voxels per cluster
NH = 2                        # halves for pipelining
GH = NSEG // NH               # 16 segments per half
CH = C_LOC // NH              # 2048 clusters per half

F32 = mybir.dt.float32
BF16 = mybir.dt.bfloat16
U8 = mybir.dt.uint8
AF = mybir.ActivationFunctionType
OP = mybir.AluOpType
AX = mybir.AxisListType

PI_2 = 1.5707963267948966
PI_6 = 0.5235987755982988

_CACHED = {}


def build_nc():
    nc = bacc.Bacc()
    xt_d = nc.dram_tensor("xt", [V, C_LOC], BF16, kind="ExternalInput").ap()
    yt_d = nc.dram_tensor("yt", [V, C_LOC], BF16, kind="ExternalInput").ap()
    zt_d = nc.dram_tensor("zt", [V, C_LOC], BF16, kind="ExternalInput").ap()
    xc_d = nc.dram_tensor("xc", [NH, P, V, GH], BF16, kind="ExternalInput").ap()
    yc_d = nc.dram_tensor("yc", [NH, P, V, GH], BF16, kind="ExternalInput").ap()
    zc_d = nc.dram_tensor("zc", [NH, P, V, GH], BF16, kind="ExternalInput").ap()
    feats_d = nc.dram_tensor("feats", [NSEG, P, 16], F32, kind="ExternalOutput").ap()

    with tile.TileContext(nc) as tc, ExitStack() as ctx:
        pool = ctx.enter_context(tc.tile_pool(name="main", bufs=1))
        sp = ctx.enter_context(tc.tile_pool(name="p1s", bufs=6))
        p2p = ctx.enter_context(tc.tile_pool(name="p2s", bufs=1))
        pp = ctx.enter_context(tc.tile_pool(name="psum", bufs=2, space="PSUM"))

        D = nc.vector   # DVE
        A = nc.scalar   # Activation
        G = nc.gpsimd   # Pool

        ones = pool.tile([P, 1], BF16, tag="ones")
        G.memset(ones[:], 1.0)
        bias_pi2 = pool.tile([P, 1], F32, tag="bias_pi2")
        bias_pi6 = pool.tile([P, 1], F32, tag="bias_pi6")
        G.memset(bias_pi2[:], PI_2)
        G.memset(bias_pi6[:], PI_6)

        # ---- input DMAs, split across issue queues so transfers overlap ----
        vm = {}   # (coord, half) -> [P, CH] bf16 voxel-major
        cm = {}   # (coord, half) -> [P, V, GH] bf16 cluster-major seg-inner
        for h in range(NH):
            for k, (name, d) in enumerate(
                    (("x", xt_d), ("y", yt_d), ("z", zt_d))):
                t = pool.tile([P, CH], BF16, tag=f"vm_{name}{h}", name=f"vm_{name}{h}")
                nc.sync.dma_start(t[:], d[:, h * CH:(h + 1) * CH])
                vm[(k, h)] = t
        for h in range(NH):
            eng = nc.scalar if h == 0 else nc.sync
            for k, (name, d) in enumerate(
                    (("x", xc_d), ("y", yc_d), ("z", zc_d))):
                t = pool.tile([P, V, GH], BF16, tag=f"cm_{name}{h}", name=f"cm_{name}{h}")
                eng.dma_start(t[:], d[h])
                cm[(k, h)] = t

        # ---- shared tiles / helpers ----
        ps = [pp.tile([P, 9 * GH], F32, tag=f"ps{h}", name=f"ps{h}")
              for h in range(NH)]
        moments = pool.tile([P, 9, NSEG], F32, tag="moments")
        Sx = moments[:, 0]; Sy = moments[:, 1]; Sz = moments[:, 2]
        Mxx = moments[:, 3]; Myy = moments[:, 4]; Mzz = moments[:, 5]
        Mxy = moments[:, 6]; Mxz = moments[:, 7]; Myz = moments[:, 8]

        feats = pool.tile([P, NSEG, 16], F32, tag="feats")

        def small(name, dt=F32):
            return pool.tile([P, NSEG], dt, tag=f"s_{name}", name=name)

        def ap(x):
            return x[:] if hasattr(x, "tag") else x

        def tt(eng, out, a, b, op):
            eng.tensor_tensor(ap(out), ap(a), ap(b), op)

        def ts(eng, out, in0, s1, s2=None, op0=OP.mult, op1=None):
            kw = dict(out=ap(out), in0=ap(in0), scalar1=s1, scalar2=s2, op0=op0)
            if op1 is not None:
                kw["op1"] = op1
            eng.tensor_scalar(**kw)

        def stt(eng, out, in0, s, in1, op0, op1):
            eng.scalar_tensor_tensor(out=ap(out), in0=ap(in0), scalar=s,
                                     in1=ap(in1), op0=op0, op1=op1)

        inv_s = 1.0 / V
        cxb = small("cxb", BF16); cyb = small("cyb", BF16); czb = small("czb", BF16)

        # ---- phase 1: moments via ACT/DVE/Pool products + PE column sums ----
        def colsum(h, plane, k):
            # column sums of [128, CH] plane: group g -> psum[:, k*GH+g]
            for g in range(GH):
                nc.tensor.matmul(
                    out=ps[h][:, k * GH + g: k * GH + g + 1],
                    lhsT=plane[:, g * P:(g + 1) * P],
                    rhs=ones[:, 0:1], start=True, stop=True)

        def p1(h):
            x, y, z = vm[(0, h)], vm[(1, h)], vm[(2, h)]
            colsum(h, x, 0)
            colsum(h, y, 1)
            colsum(h, z, 2)
            sq = sp.tile([P, CH], BF16, tag="p1s", name=f"sqx{h}")
            D.tensor_tensor(sq[:], x[:], x[:], OP.mult)
            colsum(h, sq, 3)
            sq = sp.tile([P, CH], BF16, tag="p1s", name=f"sqy{h}")
            A.activation(sq[:], y[:], AF.Square)
            colsum(h, sq, 4)
            sq = sp.tile([P, CH], BF16, tag="p1s", name=f"sqz{h}")
            G.tensor_tensor(sq[:], z[:], z[:], OP.mult)
            colsum(h, sq, 5)
            pr = sp.tile([P, CH], BF16, tag="p1s", name=f"cxy{h}")
            D.tensor_tensor(pr[:], x[:], y[:], OP.mult)
            colsum(h, pr, 6)
            pr = sp.tile([P, CH], BF16, tag="p1s", name=f"cxz{h}")
            G.tensor_tensor(pr[:], x[:], z[:], OP.mult)
            colsum(h, pr, 7)
            pr = sp.tile([P, CH], BF16, tag="p1s", name=f"cyz{h}")
            G.tensor_tensor(pr[:], y[:], z[:], OP.mult)
            colsum(h, pr, 8)
            D.tensor_copy(
                moments[:, :, h * GH:(h + 1) * GH],
                ps[h][:].rearrange("p (k g) -> p k g", k=9))

        def e_early(h):
            hs = slice(h * GH, (h + 1) * GH)
            ts(D, feats[:, hs, 0], Sx[:, hs], inv_s)
            ts(D, feats[:, hs, 1], Sy[:, hs], inv_s)
            ts(D, feats[:, hs, 2], Sz[:, hs], inv_s)
            D.tensor_copy(cxb[:, hs], feats[:, hs, 0])
            D.tensor_copy(cyb[:, hs], feats[:, hs, 1])
            D.tensor_copy(czb[:, hs], feats[:, hs, 2])

        p1(0)
        e_early(0)
        p1(1)
        e_early(1)

        # ---- phase 2a (needs only the centers from the eigen stage) ----
        def bc(t, h):
            return t[:, None, h * GH:(h + 1) * GH].broadcast_to([P, V, GH])

        Xc = [None] * NH; Yc = [None] * NH; Zc = [None] * NH; Ssum = [None] * NH

        def p2a(h):
            x, y, z = cm[(0, h)], cm[(1, h)], cm[(2, h)]
            Xc[h] = p2p.tile([P, V, GH], BF16, tag=f"Xc{h}", name=f"Xc{h}")
            Yc[h] = p2p.tile([P, V, GH], BF16, tag=f"Yc{h}", name=f"Yc{h}")
            Zc[h] = p2p.tile([P, V, GH], BF16, tag=f"Zc{h}", name=f"Zc{h}")
            G.tensor_tensor(Xc[h][:], x[:], bc(cxb, h), OP.subtract)
            G.tensor_tensor(Yc[h][:], y[:], bc(cyb, h), OP.subtract)
            G.tensor_tensor(Zc[h][:], z[:], bc(czb, h), OP.subtract)
            sx = p2p.tile([P, V, GH], BF16, tag="sx_s", name=f"sx{h}")
            sy = p2p.tile([P, V, GH], BF16, tag="sy_s", name=f"sy{h}")
            sz = p2p.tile([P, V, GH], BF16, tag="sz_s", name=f"sz{h}")
            A.activation(sx[:], Xc[h][:], AF.Square)
            A.activation(sy[:], Yc[h][:], AF.Square)
            D.tensor_tensor(sz[:], Zc[h][:], Zc[h][:], OP.mult)
            s1 = p2p.tile([P, V, GH], BF16, tag="s1_s", name=f"s1{h}")
            D.tensor_tensor(s1[:], sx[:], sy[:], OP.add)
            Ssum[h] = p2p.tile([P, V, GH], BF16, tag=f"s{h}", name=f"s{h}")
            G.tensor_tensor(Ssum[h][:], s1[:], sz[:], OP.add)

        p2a(0)
        p2a(1)

        # ---- eigen stage E-rest: [128, 32] fp32 analytic 3x3 eigensolve ----
        axx = small("axx"); ayy = small("ayy"); azz = small("azz")
        axy = small("axy"); axz = small("axz"); ayz = small("ayz")
        t0 = small("t0"); t1 = small("t1"); t2 = small("t2")
        t3 = small("t3"); t4 = small("t4"); t5 = small("t5")
        tt(D, t0, Sx, Sx, OP.mult)
        stt(D, axx, t0, -inv_s, Mxx, OP.mult, OP.add)
        tt(D, t1, Sy, Sy, OP.mult)
        stt(D, ayy, t1, -inv_s, Myy, OP.mult, OP.add)
        tt(D, t2, Sz, Sz, OP.mult)
        stt(D, azz, t2, -inv_s, Mzz, OP.mult, OP.add)
        tt(D, t3, Sx, Sy, OP.mult)
        stt(D, axy, t3, -inv_s, Mxy, OP.mult, OP.add)
        tt(D, t4, Sx, Sz, OP.mult)
        stt(D, axz, t4, -inv_s, Mxz, OP.mult, OP.add)
        tt(D, t5, Sy, Sz, OP.mult)
        stt(D, ayz, t5, -inv_s, Myz, OP.mult, OP.add)

        q = small("q")
        tt(D, t0, axx, ayy, OP.add)
        tt(D, t0, t0, azz, OP.add)
        ts(D, q, t0, 1.0 / 3.0)
        b11 = small("b11"); b22 = small("b22"); b33 = small("b33")
        tt(D, b11, axx, q, OP.subtract)
        tt(D, b22, ayy, q, OP.subtract)
        tt(D, b33, azz, q, OP.subtract)

        tt(G, t0, b11, b11, OP.mult)
        tt(G, t1, b22, b22, OP.mult)
        tt(G, t2, b33, b33, OP.mult)
        tt(G, t3, axy, axy, OP.mult)
        tt(G, t4, axz, axz, OP.mult)
        tt(G, t5, ayz, ayz, OP.mult)
        tt(D, t0, t0, t1, OP.add)
        tt(D, t0, t0, t2, OP.add)
        tt(G, t3, t3, t4, OP.add)
        tt(G, t3, t3, t5, OP.add)
        p2t = small("p2t")
        stt(D, p2t, t3, 2.0, t0, OP.mult, OP.add)
        p_ = small("p_")
        A.activation(p_[:], p2t[:], AF.Sqrt, scale=1.0 / 6.0)
        invp = small("invp")
        D.reciprocal(invp[:], p_[:])

        c11 = small("c11"); c22 = small("c22"); c33 = small("c33")
        c12 = small("c12"); c13 = small("c13"); c23 = small("c23")
        tt(D, c11, b11, invp, OP.mult)
        tt(D, c22, b22, invp, OP.mult)
        tt(D, c33, b33, invp, OP.mult)
        tt(G, c12, axy, invp, OP.mult)
        tt(G, c13, axz, invp, OP.mult)
        tt(G, c23, ayz, invp, OP.mult)

        # det(C)/2 -> r, clamped to [-1, 1]
        tt(D, t0, c22, c33, OP.mult)
        tt(G, t1, c23, c23, OP.mult)
        tt(D, t0, t0, t1, OP.subtract)
        tt(D, t0, t0, c11, OP.mult)
        tt(G, t2, c12, c33, OP.mult)
        tt(G, t3, c23, c13, OP.mult)
        tt(G, t2, t2, t3, OP.subtract)
        tt(G, t2, t2, c12, OP.mult)
        tt(D, t4, c12, c23, OP.mult)
        tt(D, t5, c22, c13, OP.mult)
        tt(D, t4, t4, t5, OP.subtract)
        tt(D, t4, t4, c13, OP.mult)
        tt(D, t0, t0, t2, OP.subtract)
        tt(D, t0, t0, t4, OP.add)
        r = small("r")
        ts(D, r, t0, 0.5, 1.0, OP.mult, OP.min)
        ts(D, r, r, -1.0, None, OP.max)

        # theta/4 = arctan(sqrt((1-r)/2) / (1 + sqrt((1+r)/2)))
        ts(D, t0, r, -0.5, 0.5, OP.mult, OP.add)
        ts(D, t1, r, 0.5, 0.5, OP.mult, OP.add)
        sa = small("sa"); sb = small("sb")
        A.activation(sa[:], t0[:], AF.Sqrt)
        A.activation(sb[:], t1[:], AF.Sqrt)
        ts(D, sb, sb, 1.0, None, OP.add)
        D.reciprocal(t2[:], sb[:])
        tt(D, t3, sa, t2, OP.mult)
        at4 = small("at4")
        A.activation(at4[:], t3[:], AF.Arctan)
        cmax = small("cmax"); smin = small("smin")
        A.activation(cmax[:], at4[:], AF.Sin, bias=bias_pi2[:, 0:1], scale=-4.0 / 3.0)
        A.activation(smin[:], at4[:], AF.Sin, bias=bias_pi6[:, 0:1], scale=4.0 / 3.0)

        w3 = small("w3"); w2 = small("w2")
        tt(D, t0, p_, cmax, OP.mult)
        stt(D, w3, t0, 2.0, q, OP.mult, OP.add)
        tt(G, t1, p_, smin, OP.mult)
        stt(D, t1, t1, -2.0, q, OP.mult, OP.add)      # w1
        stt(D, t2, q, 3.0, w3, OP.mult, OP.subtract)  # 3q - w3
        tt(D, w2, t2, t1, OP.subtract)
        invw3 = small("invw3")
        D.reciprocal(invw3[:], w3[:])
        dirwt = small("dirwt")
        tt(D, t0, w2, invw3, OP.mult)
        ts(D, dirwt, t0, -1.0, 1.0, OP.mult, OP.add)

        # B = A / w3 (9 entries, symmetric) written straight into feats
        tt(D, feats[:, :, 3], axx, invw3, OP.mult)
        tt(D, feats[:, :, 4], axy, invw3, OP.mult)
        A.copy(feats[:, :, 6], feats[:, :, 4])
        tt(D, feats[:, :, 5], axz, invw3, OP.mult)
        A.copy(feats[:, :, 9], feats[:, :, 5])
        tt(D, feats[:, :, 7], ayy, invw3, OP.mult)
        tt(D, feats[:, :, 8], ayz, invw3, OP.mult)
        A.copy(feats[:, :, 10], feats[:, :, 8])
        tt(D, feats[:, :, 11], azz, invw3, OP.mult)

        # principal eigenvector: cross products of rows of (A - w3 I)
        d1 = small("d1"); d2 = small("d2"); d3 = small("d3")
        tt(D, d1, axx, w3, OP.subtract)
        tt(D, d2, ayy, w3, OP.subtract)
        tt(D, d3, azz, w3, OP.subtract)
        u1 = small("u1"); u2 = small("u2"); u3 = small("u3")
        tt(G, t0, axy, ayz, OP.mult)
        tt(G, t1, d2, axz, OP.mult)
        tt(G, u1, t0, t1, OP.subtract)
        tt(D, t2, axy, axz, OP.mult)
        tt(D, t3, d1, ayz, OP.mult)
        tt(D, u2, t2, t3, OP.subtract)
        tt(G, t4, d1, d2, OP.mult)
        tt(G, t5, axy, axy, OP.mult)
        tt(G, u3, t4, t5, OP.subtract)
        k1 = small("k1"); k2 = small("k2")
        tt(D, t0, d2, d3, OP.mult)
        tt(D, t1, ayz, ayz, OP.mult)
        tt(D, k1, t0, t1, OP.subtract)
        tt(G, t2, ayz, axz, OP.mult)
        tt(G, t3, axy, d3, OP.mult)
        tt(G, k2, t2, t3, OP.subtract)
        # k3 = u1 (same formula)
        nu = small("nu"); nk = small("nk")
        nu1 = small("nu1")
        tt(D, nu1, u1, u1, OP.mult)
        tt(D, t0, u2, u2, OP.mult)
        tt(D, t1, u3, u3, OP.mult)
        tt(D, t0, t0, t1, OP.add)
        tt(D, nu, t0, nu1, OP.add)
        tt(G, t2, k1, k1, OP.mult)
        tt(G, t3, k2, k2, OP.mult)
        tt(G, t2, t2, t3, OP.add)
        tt(G, nk, t2, nu1, OP.add)
        m = small("m", U8)
        tt(D, m, nk, nu, OP.is_gt)
        e1 = small("e1"); e2 = small("e2"); e3 = small("e3"); ne = small("ne")
        D.select(e1[:], m[:], k1[:], u1[:])
        D.select(e2[:], m[:], k2[:], u2[:])
        D.select(e3[:], m[:], u1[:], u3[:])
        D.select(ne[:], m[:], nk[:], nu[:])
        rsn = small("rsn")
        A.activation(rsn[:], ne[:], AF.Sqrt)
        ts(D, rsn, rsn, 1e-30, None, OP.max)
        invn = small("invn")
        D.reciprocal(invn[:], rsn[:])
        v0x = small("v0x"); v0y = small("v0y"); v0z = small("v0z")
        tt(D, v0x, e1, invn, OP.mult)
        tt(D, v0y, e2, invn, OP.mult)
        tt(D, v0z, e3, invn, OP.mult)
        v0xb = small("v0xb", BF16); v0yb = small("v0yb", BF16)
        v0zb = small("v0zb", BF16)
        D.tensor_copy(v0xb[:], v0x[:])
        D.tensor_copy(v0yb[:], v0y[:])
        D.tensor_copy(v0zb[:], v0z[:])

        # ---- phase 2b: projections, residual norms, sign criterion ----
        sc = small("sc")
        G.memset(feats[:, :, 15], float(V))

        def p2b_steps(h):
            a1 = p2p.tile([P, V, GH], BF16, tag=f"a1{h}", name=f"a1{h}")
            a2 = p2p.tile([P, V, GH], BF16, tag=f"a2{h}", name=f"a2{h}")
            a3 = p2p.tile([P, V, GH], BF16, tag=f"a3{h}", name=f"a3{h}")
            yield lambda: D.tensor_tensor(a1[:], Xc[h][:], bc(v0xb, h), OP.mult)
            yield lambda: G.tensor_tensor(a2[:], Yc[h][:], bc(v0yb, h), OP.mult)
            yield lambda: D.tensor_tensor(a3[:], Zc[h][:], bc(v0zb, h), OP.mult)
            x0 = p2p.tile([P, V, GH], BF16, tag=f"x0{h}", name=f"x0{h}")
            yield lambda: D.tensor_tensor(x0[:], a1[:], a2[:], OP.add)
            yield lambda: D.tensor_tensor(x0[:], x0[:], a3[:], OP.add)
            q2 = p2p.tile([P, V, GH], BF16, tag=f"q2{h}", name=f"q2{h}")
            yield lambda: A.activation(q2[:], x0[:], AF.Square)
            n2 = p2p.tile([P, V, GH], BF16, tag=f"n2{h}", name=f"n2{h}")
            yield lambda: G.tensor_tensor(n2[:], Ssum[h][:], q2[:], OP.subtract)
            yield lambda: ts(D, n2, n2, 0.0, None, OP.max)
            np0 = p2p.tile([P, V, GH], BF16, tag=f"np0{h}", name=f"np0{h}")
            yield lambda: A.activation(np0[:], n2[:], AF.Sqrt)
            pr = p2p.tile([P, V, GH], BF16, tag=f"pr{h}", name=f"pr{h}")
            yield lambda: D.tensor_tensor(pr[:], x0[:], np0[:], OP.mult)
            yield lambda: D.tensor_reduce(sc[:, h * GH:(h + 1) * GH],
                                          pr[:].rearrange("p v g -> p g v"),
                                          axis=AX.X, op=OP.add)
            hs = slice(h * GH, (h + 1) * GH)
            yield lambda: ts(D, t0[:, hs], sc[:, hs], 0.0, -2.0, OP.is_lt, OP.mult)
            yield lambda: ts(D, t0[:, hs], t0[:, hs], 1.0, None, OP.add)
            yield lambda: tt(D, t1[:, hs], t0[:, hs], dirwt[:, hs], OP.mult)
            yield lambda: tt(D, feats[:, hs, 12], v0x[:, hs], t1[:, hs], OP.mult)
            yield lambda: tt(D, feats[:, hs, 13], v0y[:, hs], t1[:, hs], OP.mult)
            yield lambda: tt(D, feats[:, hs, 14], v0z[:, hs], t1[:, hs], OP.mult)
            yield lambda: nc.sync.dma_start(
                feats_d[h * GH:(h + 1) * GH].rearrange("g q f -> q g f"),
                feats[:, hs, :])

        # zipper the two halves so their serial chains overlap
        gens = [p2b_steps(0), p2b_steps(1)]
        done = [False, False]
        while not all(done):
            for i, g in enumerate(gens):
                if done[i]:
                    continue
                try:
                    next(g)()
                except StopIteration:
                    done[i] = True

    if not nc.is_finalized():
        nc.finalize()
    return nc


def kernel(data: np.ndarray, clusts: np.ndarray) -> np.ndarray:
    import ml_dtypes
    data = np.asarray(data, dtype=np.float32)
    clusts_np = np.asarray(clusts)
    C, S = clusts_np.shape
    assert (C, S) == (N_CLUSTS, CLUST_SIZE), (C, S)

    vox = data[:, 1:4]
    g3 = vox[clusts_np.reshape(-1).astype(np.int64)].reshape(C, S, 3)
    g3 = g3.astype(ml_dtypes.bfloat16)

    if "nc" not in _CACHED:
        _CACHED["nc"] = build_nc()
    nc = _CACHED["nc"]

    in_maps = []
    for c in range(N_CORES):
        a = g3[c * C_LOC:(c + 1) * C_LOC]          # [4096, 128, 3]
        vmt = np.ascontiguousarray(a.transpose(1, 0, 2))  # [128 vox, 4096, 3]
        # cluster-major seg-inner: [h, q, v, g] with c = (h*GH+g)*128 + q
        b = a.reshape(NH, GH, P, V, 3).transpose(0, 2, 3, 1, 4)
        b = np.ascontiguousarray(b)                # [2, 128, 128, 16, 3]
        in_maps.append({
            "xt": np.ascontiguousarray(vmt[:, :, 0]),
            "yt": np.ascontiguousarray(vmt[:, :, 1]),
            "zt": np.ascontiguousarray(vmt[:, :, 2]),
            "xc": np.ascontiguousarray(b[..., 0]),
            "yc": np.ascontiguousarray(b[..., 1]),
            "zc": np.ascontiguousarray(b[..., 2]),
        })

    res = run_bass_kernel_spmd(nc, in_maps, list(range(N_CORES)))
    out = np.concatenate(
        [res.results[c]["feats"].reshape(C_LOC, 16) for c in range(N_CORES)],
        axis=0)
    return out.astype(np.float32)


# revision 13
# speedup vs baseline: 1.6840x; 1.0074x over previous
"""Trainium2 Bass kernel: per-cluster PCA geometry features (segment reduce).

Problem: data [4194304, 6] f32, clusts [32768, 128] int — per cluster of 128
voxels compute: center (mean of xyz), normalized covariance B = A/lmax,
principal axis v0 scaled by dirwt = 1 - lmid/lmax with a sign fix, size.

Strategy (v4): shard the 32768 clusters across 8 NeuronCores (4096 each).
Host pre-gathers each cluster's voxel coords (pure permutation), casts to
bf16, and ships TWO layouts per core:
  voxel-major  xt/yt/zt [128 vox, 4096 clusters] — phase-1 moment sums run
    on the PE (column sums via ones-rhs matmuls, nearly free).
  cluster-major xc/yc/zc [128 part, 128 vox, 16 seg] per half — phase-2
    element ops. Segment-INNERMOST layout keeps every DVE operand's last AP
    dim stride-1 so bf16 ops hit the 2x DVE mode, including per-cluster
    broadcasts (stride-0 on the middle/voxel dim only).
Cluster c = g*128 + q maps to (partition q, segment g), matching the PE
column-sum output layout, so moments land directly where the eigensolve
([128, 32] fp32 small-tile analytic 3x3 solve, trig method) wants them.
Input DMAs are split across the SP/ACT/Pool issue queues so transfers
overlap; work is split across DVE/ACT/Pool by measured cost-model rates
(Pool subtract is cheaper than mult); ACT table switches (sqrt<->trig
sets) are batched; feature values are written straight into the output
tile; tails and output DMA run per half.
"""
import numpy as np
from contextlib import ExitStack

import concourse.bass as bass
import concourse.bacc as bacc
import concourse.tile as tile
from concourse import mybir
from concourse.bass_utils import run_bass_kernel_spmd

N_CLUSTS = 32768
CLUST_SIZE = 128
N_CORES = 8
C_LOC = N_CLUSTS // N_CORES   # 4096 clusters per core
P = 128                       # SBUF partitions
NSEG = C_LOC // P             # 32 clusters (segments) per partition
V = CLUST_SIZE                # 128 voxels per cluster
NH = 2                        # halves for pipelining
GH = NSEG // NH               # 16 segments per half
CH = C_LOC // NH              # 2048 clusters per half

F32 = mybir.dt.float32
BF16 = mybir.dt.bfloat16
U8 = mybir.dt.uint8
AF = mybir.ActivationFunctionType
OP = mybir.AluOpType
AX = mybir.AxisListType

PI_2 = 1.5707963267948966
PI_6 = 0.5235987755982988

_CACHED = {}


def build_nc():
    nc = bacc.Bacc()
    xt_d = nc.dram_tensor("xt", [V, C_LOC], BF16, kind="ExternalInput").ap()
    yt_d = nc.dram_tensor("yt", [V, C_LOC], BF16, kind="ExternalInput").ap()
    zt_d = nc.dram_tensor("zt", [V, C_LOC], BF16, kind="ExternalInput").ap()
    xc_d = nc.dram_tensor("xc", [NH, P, V, GH], BF16, kind="ExternalInput").ap()
    yc_d = nc.dram_tensor("yc", [NH, P, V, GH], BF16, kind="ExternalInput").ap()
    zc_d = nc.dram_tensor("zc", [NH, P, V, GH], BF16, kind="ExternalInput").ap()
    feats_d = nc.dram_tensor("feats", [NSEG, P, 16], F32, kind="ExternalOutput").ap()

    with tile.TileContext(nc) as tc, ExitStack() as ctx:
        pool = ctx.enter_context(tc.tile_pool(name="main", bufs=1))
        sp = ctx.enter_context(tc.tile_pool(name="p1s", bufs=6))
        p2p = ctx.enter_context(tc.tile_pool(name="p2s", bufs=1))
        pp = ctx.enter_context(tc.tile_pool(name="psum", bufs=2, space="PSUM"))

        D = nc.vector   # DVE
        A = nc.scalar   # Activation
        G = nc.gpsimd   # Pool

        ones = pool.tile([P, 1], BF16, tag="ones")
        G.memset(ones[:], 1.0)
        bias_pi2 = pool.tile([P, 1], F32, tag="bias_pi2")
        bias_pi6 = pool.tile([P, 1], F32, tag="bias_pi6")
        G.memset(bias_pi2[:], PI_2)
        G.memset(bias_pi6[:], PI_6)

        # ---- input DMAs, split across issue queues so transfers overlap ----
        vm = {}   # (coord, half) -> [P, CH] bf16 voxel-major
        cm = {}   # (coord, half) -> [P, V, GH] bf16 cluster-major seg-inner
        for h in range(NH):
            for k, (name, d) in enumerate(
                    (("x", xt_d), ("y", yt_d), ("z", zt_d))):
                t = pool.tile([P, CH], BF16, tag=f"vm_{name}{h}", name=f"vm_{name}{h}")
                nc.sync.dma_start(t[:], d[:, h * CH:(h + 1) * CH])
                vm[(k, h)] = t
        for h in range(NH):
            eng = nc.scalar if h == 0 else nc.sync
            for k, (name, d) in enumerate(
                    (("x", xc_d), ("y", yc_d), ("z", zc_d))):
                t = pool.tile([P, V, GH], BF16, tag=f"cm_{name}{h}", name=f"cm_{name}{h}")
                eng.dma_start(t[:], d[h])
                cm[(k, h)] = t

        # ---- shared tiles / helpers ----
        ps = [pp.tile([P, 9 * GH], F32, tag=f"ps{h}", name=f"ps{h}")
              for h in range(NH)]
        moments = pool.tile([P, 9, NSEG], F32, tag="moments")
        Sx = moments[:, 0]; Sy = moments[:, 1]; Sz = moments[:, 2]
        Mxx = moments[:, 3]; Myy = moments[:, 4]; Mzz = moments[:, 5]
        Mxy = moments[:, 6]; Mxz = moments[:, 7]; Myz = moments[:, 8]

        feats = pool.tile([P, NSEG, 16], F32, tag="feats")

        def small(name, dt=F32):
            return pool.tile([P, NSEG], dt, tag=f"s_{name}", name=name)

        def ap(x):
            return x[:] if hasattr(x, "tag") else x

        def tt(eng, out, a, b, op):
            eng.tensor_tensor(ap(out), ap(a), ap(b), op)

        def ts(eng, out, in0, s1, s2=None, op0=OP.mult, op1=None):
            kw = dict(out=ap(out), in0=ap(in0), scalar1=s1, scalar2=s2, op0=op0)
            if op1 is not None:
                kw["op1"] = op1
            eng.tensor_scalar(**kw)

        def stt(eng, out, in0, s, in1, op0, op1):
            eng.scalar_tensor_tensor(out=ap(out), in0=ap(in0), scalar=s,
                                     in1=ap(in1), op0=op0, op1=op1)

        inv_s = 1.0 / V
        cxb = small("cxb", BF16); cyb = small("cyb", BF16); czb = small("czb", BF16)

        # ---- phase 1: moments via ACT/DVE/Pool products + PE column sums ----
        def colsum(h, plane, k):
            # column sums of [128, CH] plane: group g -> psum[:, k*GH+g]
            for g in range(GH):
                nc.tensor.matmul(
                    out=ps[h][:, k * GH + g: k * GH + g + 1],
                    lhsT=plane[:, g * P:(g + 1) * P],
                    rhs=ones[:, 0:1], start=True, stop=True)

        def p1(h):
            x, y, z = vm[(0, h)], vm[(1, h)], vm[(2, h)]
            colsum(h, x, 0)
            colsum(h, y, 1)
            colsum(h, z, 2)
            sq = sp.tile([P, CH], BF16, tag="p1s", name=f"sqx{h}")
            D.tensor_tensor(sq[:], x[:], x[:], OP.mult)
            colsum(h, sq, 3)
            sq = sp.tile([P, CH], BF16, tag="p1s", name=f"sqy{h}")
            A.activation(sq[:], y[:], AF.Square)
            colsum(h, sq, 4)
            sq = sp.tile([P, CH], BF16, tag="p1s", name=f"sqz{h}")
            G.tensor_tensor(sq[:], z[:], z[:], OP.mult)
            colsum(h, sq, 5)
            pr = sp.tile([P, CH], BF16, tag="p1s", name=f"cxy{h}")
            D.tensor_tensor(pr[:], x[:], y[:], OP.mult)
            colsum(h, pr, 6)
            pr = sp.tile([P, CH], BF16, tag="p1s", name=f"cxz{h}")
            G.tensor_tensor(pr[:], x[:], z[:], OP.mult)
            colsum(h, pr, 7)
            pr = sp.tile([P, CH], BF16, tag="p1s", name=f"cyz{h}")
            G.tensor_tensor(pr[:], y[:], z[:], OP.mult)
            colsum(h, pr, 8)
            D.tensor_copy(
                moments[:, :, h * GH:(h + 1) * GH],
                ps[h][:].rearrange("p (k g) -> p k g", k=9))

        def e_early(h):
            hs = slice(h * GH, (h + 1) * GH)
            ts(D, feats[:, hs, 0], Sx[:, hs], inv_s)
            ts(D, feats[:, hs, 1], Sy[:, hs], inv_s)
            ts(D, feats[:, hs, 2], Sz[:, hs], inv_s)
            D.tensor_copy(cxb[:, hs], feats[:, hs, 0])
            D.tensor_copy(cyb[:, hs], feats[:, hs, 1])
            D.tensor_copy(czb[:, hs], feats[:, hs, 2])

        p1(0)
        e_early(0)
        p1(1)
        e_early(1)

        # ---- phase 2a in 4 zippered quarters (needs only the centers) ----
        NQ = 4
        GQ = NSEG // NQ   # 8 segments per quarter

        def bcq(t, qq):
            return t[:, None, qq * GQ:(qq + 1) * GQ].broadcast_to([P, V, GQ])

        Xc = [None] * NQ; Yc = [None] * NQ; Zc = [None] * NQ; Ssum = [None] * NQ

        def cmq(k, qq):
            h, r = divmod(qq, NQ // NH)
            return cm[(k, h)][:, :, r * GQ:(r + 1) * GQ]

        def p2a_steps(qq):
            Xc[qq] = p2p.tile([P, V, GQ], BF16, tag=f"Xc{qq}", name=f"Xc{qq}")
            Yc[qq] = p2p.tile([P, V, GQ], BF16, tag=f"Yc{qq}", name=f"Yc{qq}")
            Zc[qq] = p2p.tile([P, V, GQ], BF16, tag=f"Zc{qq}", name=f"Zc{qq}")
            yield lambda: G.tensor_tensor(Xc[qq][:], cmq(0, qq), bcq(cxb, qq), OP.subtract)
            yield lambda: G.tensor_tensor(Yc[qq][:], cmq(1, qq), bcq(cyb, qq), OP.subtract)
            yield lambda: G.tensor_tensor(Zc[qq][:], cmq(2, qq), bcq(czb, qq), OP.subtract)
            sx = p2p.tile([P, V, GQ], BF16, tag=f"sx{qq}", name=f"sx{qq}")
            sy = p2p.tile([P, V, GQ], BF16, tag=f"sy{qq}", name=f"sy{qq}")
            sz = p2p.tile([P, V, GQ], BF16, tag=f"sz{qq}", name=f"sz{qq}")
            yield lambda: A.activation(sx[:], Xc[qq][:], AF.Square)
            yield lambda: A.activation(sy[:], Yc[qq][:], AF.Square)
            yield lambda: D.tensor_tensor(sz[:], Zc[qq][:], Zc[qq][:], OP.mult)
            yield lambda: D.tensor_tensor(sx[:], sx[:], sy[:], OP.add)
            Ssum[qq] = p2p.tile([P, V, GQ], BF16, tag=f"s{qq}", name=f"s{qq}")
            yield lambda: G.tensor_tensor(Ssum[qq][:], sx[:], sz[:], OP.add)

        def zipper(gens):
            done = [False] * len(gens)
            while not all(done):
                for i, g in enumerate(gens):
                    if done[i]:
                        continue
                    try:
                        next(g)()
                    except StopIteration:
                        done[i] = True

        zipper([p2a_steps(qq) for qq in range(NQ)])

        # ---- eigen stage E-rest: [128, 32] fp32 analytic 3x3 eigensolve ----
        axx = small("axx"); ayy = small("ayy"); azz = small("azz")
        axy = small("axy"); axz = small("axz"); ayz = small("ayz")
        t0 = small("t0"); t1 = small("t1"); t2 = small("t2")
        t3 = small("t3"); t4 = small("t4"); t5 = small("t5")
        tt(D, t0, Sx, Sx, OP.mult)
        stt(D, axx, t0, -inv_s, Mxx, OP.mult, OP.add)
        tt(D, t1, Sy, Sy, OP.mult)
        stt(D, ayy, t1, -inv_s, Myy, OP.mult, OP.add)
        tt(D, t2, Sz, Sz, OP.mult)
        stt(D, azz, t2, -inv_s, Mzz, OP.mult, OP.add)
        tt(D, t3, Sx, Sy, OP.mult)
        stt(D, axy, t3, -inv_s, Mxy, OP.mult, OP.add)
        tt(D, t4, Sx, Sz, OP.mult)
        stt(D, axz, t4, -inv_s, Mxz, OP.mult, OP.add)
        tt(D, t5, Sy, Sz, OP.mult)
        stt(D, ayz, t5, -inv_s, Myz, OP.mult, OP.add)

        q = small("q")
        tt(D, t0, axx, ayy, OP.add)
        tt(D, t0, t0, azz, OP.add)
        ts(D, q, t0, 1.0 / 3.0)
        b11 = small("b11"); b22 = small("b22"); b33 = small("b33")
        tt(D, b11, axx, q, OP.subtract)
        tt(D, b22, ayy, q, OP.subtract)
        tt(D, b33, azz, q, OP.subtract)

        tt(G, t0, b11, b11, OP.mult)
        tt(G, t1, b22, b22, OP.mult)
        tt(G, t2, b33, b33, OP.mult)
        tt(G, t3, axy, axy, OP.mult)
        tt(G, t4, axz, axz, OP.mult)
        tt(G, t5, ayz, ayz, OP.mult)
        tt(D, t0, t0, t1, OP.add)
        tt(D, t0, t0, t2, OP.add)
        tt(G, t3, t3, t4, OP.add)
        tt(G, t3, t3, t5, OP.add)
        p2t = small("p2t")
        stt(D, p2t, t3, 2.0, t0, OP.mult, OP.add)
        p_ = small("p_")
        A.activation(p_[:], p2t[:], AF.Sqrt, scale=1.0 / 6.0)
        invp = small("invp")
        D.reciprocal(invp[:], p_[:])

        c11 = small("c11"); c22 = small("c22"); c33 = small("c33")
        c12 = small("c12"); c13 = small("c13"); c23 = small("c23")
        tt(D, c11, b11, invp, OP.mult)
        tt(D, c22, b22, invp, OP.mult)
        tt(D, c33, b33, invp, OP.mult)
        tt(G, c12, axy, invp, OP.mult)
        tt(G, c13, axz, invp, OP.mult)
        tt(G, c23, ayz, invp, OP.mult)

        # det(C)/2 -> r, clamped to [-1, 1]
        tt(D, t0, c22, c33, OP.mult)
        tt(G, t1, c23, c23, OP.mult)
        tt(D, t0, t0, t1, OP.subtract)
        tt(D, t0, t0, c11, OP.mult)
        tt(G, t2, c12, c33, OP.mult)
        tt(G, t3, c23, c13, OP.mult)
        tt(G, t2, t2, t3, OP.subtract)
        tt(G, t2, t2, c12, OP.mult)
        tt(D, t4, c12, c23, OP.mult)
        tt(D, t5, c22, c13, OP.mult)
        tt(D, t4, t4, t5, OP.subtract)
        tt(D, t4, t4, c13, OP.mult)
        tt(D, t0, t0, t2, OP.subtract)
        tt(D, t0, t0, t4, OP.add)
        r = small("r")
        ts(D, r, t0, 0.5, 1.0, OP.mult, OP.min)
        ts(D, r, r, -1.0, None, OP.max)

        # theta/4 = arctan(sqrt((1-r)/2) / (1 + sqrt((1+r)/2)))
        ts(D, t0, r, -0.5, 0.5, OP.mult, OP.add)
        ts(D, t1, r, 0.5, 0.5, OP.mult, OP.add)
        sa = small("sa"); sb = small("sb")
        A.activation(sa[:], t0[:], AF.Sqrt)
        A.activation(sb[:], t1[:], AF.Sqrt)
        ts(D, sb, sb, 1.0, None, OP.add)
        D.reciprocal(t2[:], sb[:])
        tt(D, t3, sa, t2, OP.mult)
        at4 = small("at4")
        A.activation(at4[:], t3[:], AF.Arctan)
        cmax = small("cmax"); smin = small("smin")
        A.activation(cmax[:], at4[:], AF.Sin, bias=bias_pi2[:, 0:1], scale=-4.0 / 3.0)
        A.activation(smin[:], at4[:], AF.Sin, bias=bias_pi6[:, 0:1], scale=4.0 / 3.0)

        w3 = small("w3"); w2 = small("w2")
        tt(D, t0, p_, cmax, OP.mult)
        stt(D, w3, t0, 2.0, q, OP.mult, OP.add)
        tt(G, t1, p_, smin, OP.mult)
        stt(D, t1, t1, -2.0, q, OP.mult, OP.add)      # w1
        stt(D, t2, q, 3.0, w3, OP.mult, OP.subtract)  # 3q - w3
        tt(D, w2, t2, t1, OP.subtract)
        invw3 = small("invw3")
        D.reciprocal(invw3[:], w3[:])
        dirwt = small("dirwt")
        tt(D, t0, w2, invw3, OP.mult)
        ts(D, dirwt, t0, -1.0, 1.0, OP.mult, OP.add)

        # B = A / w3 (9 entries, symmetric) written straight into feats
        tt(D, feats[:, :, 3], axx, invw3, OP.mult)
        tt(D, feats[:, :, 4], axy, invw3, OP.mult)
        A.copy(feats[:, :, 6], feats[:, :, 4])
        tt(D, feats[:, :, 5], axz, invw3, OP.mult)
        A.copy(feats[:, :, 9], feats[:, :, 5])
        tt(D, feats[:, :, 7], ayy, invw3, OP.mult)
        tt(D, feats[:, :, 8], ayz, invw3, OP.mult)
        A.copy(feats[:, :, 10], feats[:, :, 8])
        tt(D, feats[:, :, 11], azz, invw3, OP.mult)

        # principal eigenvector: cross products of rows of (A - w3 I)
        d1 = small("d1"); d2 = small("d2"); d3 = small("d3")
        tt(D, d1, axx, w3, OP.subtract)
        tt(D, d2, ayy, w3, OP.subtract)
        tt(D, d3, azz, w3, OP.subtract)
        u1 = small("u1"); u2 = small("u2"); u3 = small("u3")
        tt(G, t0, axy, ayz, OP.mult)
        tt(G, t1, d2, axz, OP.mult)
        tt(G, u1, t0, t1, OP.subtract)
        tt(D, t2, axy, axz, OP.mult)
        tt(D, t3, d1, ayz, OP.mult)
        tt(D, u2, t2, t3, OP.subtract)
        tt(G, t4, d1, d2, OP.mult)
        tt(G, t5, axy, axy, OP.mult)
        tt(G, u3, t4, t5, OP.subtract)
        k1 = small("k1"); k2 = small("k2")
        tt(D, t0, d2, d3, OP.mult)
        tt(D, t1, ayz, ayz, OP.mult)
        tt(D, k1, t0, t1, OP.subtract)
        tt(G, t2, ayz, axz, OP.mult)
        tt(G, t3, axy, d3, OP.mult)
        tt(G, k2, t2, t3, OP.subtract)
        # k3 = u1 (same formula)
        nu = small("nu"); nk = small("nk")
        nu1 = small("nu1")
        tt(D, nu1, u1, u1, OP.mult)
        tt(D, t0, u2, u2, OP.mult)
        tt(D, t1, u3, u3, OP.mult)
        tt(D, t0, t0, t1, OP.add)
        tt(D, nu, t0, nu1, OP.add)
        tt(G, t2, k1, k1, OP.mult)
        tt(G, t3, k2, k2, OP.mult)
        tt(G, t2, t2, t3, OP.add)
        tt(G, nk, t2, nu1, OP.add)
        m = small("m", U8)
        tt(D, m, nk, nu, OP.is_gt)
        e1 = small("e1"); e2 = small("e2"); e3 = small("e3"); ne = small("ne")
        D.select(e1[:], m[:], k1[:], u1[:])
        D.select(e2[:], m[:], k2[:], u2[:])
        D.select(e3[:], m[:], u1[:], u3[:])
        D.select(ne[:], m[:], nk[:], nu[:])
        rsn = small("rsn")
        A.activation(rsn[:], ne[:], AF.Sqrt)
        ts(D, rsn, rsn, 1e-30, None, OP.max)
        invn = small("invn")
        D.reciprocal(invn[:], rsn[:])
        v0x = small("v0x"); v0y = small("v0y"); v0z = small("v0z")
        tt(D, v0x, e1, invn, OP.mult)
        tt(D, v0y, e2, invn, OP.mult)
        tt(D, v0z, e3, invn, OP.mult)
        v0xb = small("v0xb", BF16); v0yb = small("v0yb", BF16)
        v0zb = small("v0zb", BF16)
        D.tensor_copy(v0xb[:], v0x[:])
        D.tensor_copy(v0yb[:], v0y[:])
        D.tensor_copy(v0zb[:], v0z[:])

        # ---- phase 2b: projections, residual norms, sign criterion ----
        sc = small("sc")
        G.memset(feats[:, :, 15], float(V))

        def p2b_steps(qq):
            a1 = p2p.tile([P, V, GQ], BF16, tag=f"a1{qq}", name=f"a1{qq}")
            a2 = p2p.tile([P, V, GQ], BF16, tag=f"a2{qq}", name=f"a2{qq}")
            a3 = p2p.tile([P, V, GQ], BF16, tag=f"a3{qq}", name=f"a3{qq}")
            yield lambda: D.tensor_tensor(a1[:], Xc[qq][:], bcq(v0xb, qq), OP.mult)
            yield lambda: G.tensor_tensor(a2[:], Yc[qq][:], bcq(v0yb, qq), OP.mult)
            yield lambda: D.tensor_tensor(a3[:], Zc[qq][:], bcq(v0zb, qq), OP.mult)
            x0 = p2p.tile([P, V, GQ], BF16, tag=f"x0{qq}", name=f"x0{qq}")
            yield lambda: D.tensor_tensor(x0[:], a1[:], a2[:], OP.add)
            yield lambda: D.tensor_tensor(x0[:], x0[:], a3[:], OP.add)
            q2 = p2p.tile([P, V, GQ], BF16, tag=f"q2{qq}", name=f"q2{qq}")
            yield lambda: A.activation(q2[:], x0[:], AF.Square)
            yield lambda: G.tensor_tensor(q2[:], Ssum[qq][:], q2[:], OP.subtract)
            yield lambda: ts(D, q2, q2, 0.0, None, OP.max)
            yield lambda: A.activation(q2[:], q2[:], AF.Sqrt)
            yield lambda: D.tensor_tensor(x0[:], x0[:], q2[:], OP.mult)
            yield lambda: D.tensor_reduce(sc[:, qq * GQ:(qq + 1) * GQ],
                                          x0[:].rearrange("p v g -> p g v"),
                                          axis=AX.X, op=OP.add)
            qs = slice(qq * GQ, (qq + 1) * GQ)
            yield lambda: ts(D, t0[:, qs], sc[:, qs], 0.0, -2.0, OP.is_lt, OP.mult)
            yield lambda: ts(D, t0[:, qs], t0[:, qs], 1.0, None, OP.add)
            yield lambda: tt(D, t1[:, qs], t0[:, qs], dirwt[:, qs], OP.mult)
            yield lambda: tt(D, feats[:, qs, 12], v0x[:, qs], t1[:, qs], OP.mult)
            yield lambda: tt(D, feats[:, qs, 13], v0y[:, qs], t1[:, qs], OP.mult)
            yield lambda: tt(D, feats[:, qs, 14], v0z[:, qs], t1[:, qs], OP.mult)
            yield lambda: nc.sync.dma_start(
                feats_d[qq * GQ:(qq + 1) * GQ].rearrange("g q f -> q g f"),
                feats[:, qs, :])

        zipper([p2b_steps(qq) for qq in range(NQ)])

    if not nc.is_finalized():
        nc.finalize()
    return nc


def kernel(data: np.ndarray, clusts: np.ndarray) -> np.ndarray:
    import ml_dtypes
    data = np.asarray(data, dtype=np.float32)
    clusts_np = np.asarray(clusts)
    C, S = clusts_np.shape
    assert (C, S) == (N_CLUSTS, CLUST_SIZE), (C, S)

    vox = data[:, 1:4]
    g3 = vox[clusts_np.reshape(-1).astype(np.int64)].reshape(C, S, 3)
    g3 = g3.astype(ml_dtypes.bfloat16)

    if "nc" not in _CACHED:
        _CACHED["nc"] = build_nc()
    nc = _CACHED["nc"]

    in_maps = []
    for c in range(N_CORES):
        a = g3[c * C_LOC:(c + 1) * C_LOC]          # [4096, 128, 3]
        vmt = np.ascontiguousarray(a.transpose(1, 0, 2))  # [128 vox, 4096, 3]
        # cluster-major seg-inner: [h, q, v, g] with c = (h*GH+g)*128 + q
        b = a.reshape(NH, GH, P, V, 3).transpose(0, 2, 3, 1, 4)
        b = np.ascontiguousarray(b)                # [2, 128, 128, 16, 3]
        in_maps.append({
            "xt": np.ascontiguousarray(vmt[:, :, 0]),
            "yt": np.ascontiguousarray(vmt[:, :, 1]),
            "zt": np.ascontiguousarray(vmt[:, :, 2]),
            "xc": np.ascontiguousarray(b[..., 0]),
            "yc": np.ascontiguousarray(b[..., 1]),
            "zc": np.ascontiguousarray(b[..., 2]),
        })

    res = run_bass_kernel_spmd(nc, in_maps, list(range(N_CORES)))
    out = np.concatenate(
        [res.results[c]["feats"].reshape(C_LOC, 16) for c in range(N_CORES)],
        axis=0)
    return out.astype(np.float32)


# revision 14
# speedup vs baseline: 1.7626x; 1.0467x over previous
"""Trainium2 Bass kernel: per-cluster PCA geometry features (segment reduce).

Problem: data [4194304, 6] f32, clusts [32768, 128] int — per cluster of 128
voxels compute: center (mean of xyz), normalized covariance B = A/lmax,
principal axis v0 scaled by dirwt = 1 - lmid/lmax with a sign fix, size.

Strategy (v4): shard the 32768 clusters across 8 NeuronCores (4096 each).
Host pre-gathers each cluster's voxel coords (pure permutation), casts to
bf16, and ships TWO layouts per core:
  voxel-major  xt/yt/zt [128 vox, 4096 clusters] — phase-1 moment sums run
    on the PE (column sums via ones-rhs matmuls, nearly free).
  cluster-major xc/yc/zc [128 part, 128 vox, 16 seg] per half — phase-2
    element ops. Segment-INNERMOST layout keeps every DVE operand's last AP
    dim stride-1 so bf16 ops hit the 2x DVE mode, including per-cluster
    broadcasts (stride-0 on the middle/voxel dim only).
Cluster c = g*128 + q maps to (partition q, segment g), matching the PE
column-sum output layout, so moments land directly where the eigensolve
([128, 32] fp32 small-tile analytic 3x3 solve, trig method) wants them.
Input DMAs are split across the SP/ACT/Pool issue queues so transfers
overlap; work is split across DVE/ACT/Pool by measured cost-model rates
(Pool subtract is cheaper than mult); ACT table switches (sqrt<->trig
sets) are batched; feature values are written straight into the output
tile; tails and output DMA run per half.
"""
import numpy as np
from contextlib import ExitStack

import concourse.bass as bass
import concourse.bacc as bacc
import concourse.tile as tile
from concourse import mybir
from concourse.bass_utils import run_bass_kernel_spmd

N_CLUSTS = 32768
CLUST_SIZE = 128
N_CORES = 8
C_LOC = N_CLUSTS // N_CORES   # 4096 clusters per core
P = 128                       # SBUF partitions
NSEG = C_LOC // P             # 32 clusters (segments) per partition
V = CLUST_SIZE                # 128 voxels per cluster
NH = 2                        # halves for pipelining
GH = NSEG // NH               # 16 segments per half
CH = C_LOC // NH              # 2048 clusters per half

F32 = mybir.dt.float32
BF16 = mybir.dt.bfloat16
U8 = mybir.dt.uint8
AF = mybir.ActivationFunctionType
OP = mybir.AluOpType
AX = mybir.AxisListType

PI_2 = 1.5707963267948966
PI_6 = 0.5235987755982988

_CACHED = {}


def build_nc():
    nc = bacc.Bacc()
    xt_d = nc.dram_tensor("xt", [V, C_LOC], BF16, kind="ExternalInput").ap()
    yt_d = nc.dram_tensor("yt", [V, C_LOC], BF16, kind="ExternalInput").ap()
    zt_d = nc.dram_tensor("zt", [V, C_LOC], BF16, kind="ExternalInput").ap()
    xc_d = nc.dram_tensor("xc", [NH, P, V, GH], BF16, kind="ExternalInput").ap()
    yc_d = nc.dram_tensor("yc", [NH, P, V, GH], BF16, kind="ExternalInput").ap()
    zc_d = nc.dram_tensor("zc", [NH, P, V, GH], BF16, kind="ExternalInput").ap()
    feats_d = nc.dram_tensor("feats", [NSEG, P, 16], F32, kind="ExternalOutput").ap()

    with tile.TileContext(nc) as tc, ExitStack() as ctx:
        pool = ctx.enter_context(tc.tile_pool(name="main", bufs=1))
        sp = ctx.enter_context(tc.tile_pool(name="p1s", bufs=6))
        p2p = ctx.enter_context(tc.tile_pool(name="p2s", bufs=1))
        pp = ctx.enter_context(tc.tile_pool(name="psum", bufs=2, space="PSUM"))

        D = nc.vector   # DVE
        A = nc.scalar   # Activation
        G = nc.gpsimd   # Pool

        ones = pool.tile([P, 1], BF16, tag="ones")
        G.memset(ones[:], 1.0)
        bias_pi2 = pool.tile([P, 1], F32, tag="bias_pi2")
        bias_pi6 = pool.tile([P, 1], F32, tag="bias_pi6")
        G.memset(bias_pi2[:], PI_2)
        G.memset(bias_pi6[:], PI_6)
        A.activation(bias_pi6[:], bias_pi2[:], AF.Sqrt)
        G.memset(bias_pi6[:], PI_6)

        # ---- input DMAs, split across issue queues so transfers overlap ----
        vm = {}   # (coord, half) -> [P, CH] bf16 voxel-major
        cm = {}   # (coord, half) -> [P, V, GH] bf16 cluster-major seg-inner
        for h in range(NH):
            veng = nc.sync if h == 0 else nc.gpsimd
            for k, (name, d) in enumerate(
                    (("x", xt_d), ("y", yt_d), ("z", zt_d))):
                t = pool.tile([P, CH], BF16, tag=f"vm_{name}{h}", name=f"vm_{name}{h}")
                veng.dma_start(t[:], d[:, h * CH:(h + 1) * CH])
                vm[(k, h)] = t
        for h in range(NH):
            eng = nc.scalar if h == 0 else nc.sync
            for k, (name, d) in enumerate(
                    (("x", xc_d), ("y", yc_d), ("z", zc_d))):
                t = pool.tile([P, V, GH], BF16, tag=f"cm_{name}{h}", name=f"cm_{name}{h}")
                eng.dma_start(t[:], d[h])
                cm[(k, h)] = t

        # ---- shared tiles / helpers ----
        ps = [pp.tile([P, 9 * GH], F32, tag=f"ps{h}", name=f"ps{h}")
              for h in range(NH)]
        moments = pool.tile([P, 9, NSEG], F32, tag="moments")
        Sx = moments[:, 0]; Sy = moments[:, 1]; Sz = moments[:, 2]
        Mxx = moments[:, 3]; Myy = moments[:, 4]; Mzz = moments[:, 5]
        Mxy = moments[:, 6]; Mxz = moments[:, 7]; Myz = moments[:, 8]

        feats = pool.tile([P, NSEG, 16], F32, tag="feats")

        def small(name, dt=F32):
            return pool.tile([P, NSEG], dt, tag=f"s_{name}", name=name)

        def ap(x):
            return x[:] if hasattr(x, "tag") else x

        def tt(eng, out, a, b, op):
            eng.tensor_tensor(ap(out), ap(a), ap(b), op)

        def ts(eng, out, in0, s1, s2=None, op0=OP.mult, op1=None):
            kw = dict(out=ap(out), in0=ap(in0), scalar1=s1, scalar2=s2, op0=op0)
            if op1 is not None:
                kw["op1"] = op1
            eng.tensor_scalar(**kw)

        def stt(eng, out, in0, s, in1, op0, op1):
            eng.scalar_tensor_tensor(out=ap(out), in0=ap(in0), scalar=s,
                                     in1=ap(in1), op0=op0, op1=op1)

        inv_s = 1.0 / V
        cxb = small("cxb", BF16); cyb = small("cyb", BF16); czb = small("czb", BF16)

        # ---- phase 1: moments via ACT/DVE/Pool products + PE column sums ----
        def colsum(h, plane, k):
            # column sums of [128, CH] plane: group g -> psum[:, k*GH+g]
            for g in range(GH):
                nc.tensor.matmul(
                    out=ps[h][:, k * GH + g: k * GH + g + 1],
                    lhsT=plane[:, g * P:(g + 1) * P],
                    rhs=ones[:, 0:1], start=True, stop=True)

        def p1(h):
            x, y, z = vm[(0, h)], vm[(1, h)], vm[(2, h)]
            colsum(h, x, 0)
            colsum(h, y, 1)
            colsum(h, z, 2)
            sq = sp.tile([P, CH], BF16, tag="p1s", name=f"sqx{h}")
            D.tensor_tensor(sq[:], x[:], x[:], OP.mult)
            colsum(h, sq, 3)
            sq = sp.tile([P, CH], BF16, tag="p1s", name=f"sqy{h}")
            A.activation(sq[:], y[:], AF.Square)
            colsum(h, sq, 4)
            sq = sp.tile([P, CH], BF16, tag="p1s", name=f"sqz{h}")
            G.tensor_tensor(sq[:], z[:], z[:], OP.mult)
            colsum(h, sq, 5)
            pr = sp.tile([P, CH], BF16, tag="p1s", name=f"cxy{h}")
            D.tensor_tensor(pr[:], x[:], y[:], OP.mult)
            colsum(h, pr, 6)
            pr = sp.tile([P, CH], BF16, tag="p1s", name=f"cxz{h}")
            G.tensor_tensor(pr[:], x[:], z[:], OP.mult)
            colsum(h, pr, 7)
            pr = sp.tile([P, CH], BF16, tag="p1s", name=f"cyz{h}")
            G.tensor_tensor(pr[:], y[:], z[:], OP.mult)
            colsum(h, pr, 8)
            D.tensor_copy(
                moments[:, :, h * GH:(h + 1) * GH],
                ps[h][:].rearrange("p (k g) -> p k g", k=9))

        def e_early(h):
            hs = slice(h * GH, (h + 1) * GH)
            ts(D, feats[:, hs, 0], Sx[:, hs], inv_s)
            ts(D, feats[:, hs, 1], Sy[:, hs], inv_s)
            ts(D, feats[:, hs, 2], Sz[:, hs], inv_s)
            D.tensor_copy(cxb[:, hs], feats[:, hs, 0])
            D.tensor_copy(cyb[:, hs], feats[:, hs, 1])
            D.tensor_copy(czb[:, hs], feats[:, hs, 2])

        p1(0)
        e_early(0)
        p1(1)
        e_early(1)

        # ---- phase 2a in 4 zippered quarters (needs only the centers) ----
        NQ = 4
        GQ = NSEG // NQ   # 8 segments per quarter

        def bcq(t, qq):
            return t[:, None, qq * GQ:(qq + 1) * GQ].broadcast_to([P, V, GQ])

        Xc = [None] * NQ; Yc = [None] * NQ; Zc = [None] * NQ; Ssum = [None] * NQ

        def cmq(k, qq):
            h, r = divmod(qq, NQ // NH)
            return cm[(k, h)][:, :, r * GQ:(r + 1) * GQ]

        def p2a_steps(qq):
            Xc[qq] = p2p.tile([P, V, GQ], BF16, tag=f"Xc{qq}", name=f"Xc{qq}")
            Yc[qq] = p2p.tile([P, V, GQ], BF16, tag=f"Yc{qq}", name=f"Yc{qq}")
            Zc[qq] = p2p.tile([P, V, GQ], BF16, tag=f"Zc{qq}", name=f"Zc{qq}")
            yield lambda: G.tensor_tensor(Xc[qq][:], cmq(0, qq), bcq(cxb, qq), OP.subtract)
            yield lambda: G.tensor_tensor(Yc[qq][:], cmq(1, qq), bcq(cyb, qq), OP.subtract)
            yield lambda: G.tensor_tensor(Zc[qq][:], cmq(2, qq), bcq(czb, qq), OP.subtract)
            sx = p2p.tile([P, V, GQ], BF16, tag=f"sx{qq}", name=f"sx{qq}")
            sy = p2p.tile([P, V, GQ], BF16, tag=f"sy{qq}", name=f"sy{qq}")
            sz = p2p.tile([P, V, GQ], BF16, tag=f"sz{qq}", name=f"sz{qq}")
            yield lambda: A.activation(sx[:], Xc[qq][:], AF.Square)
            yield lambda: A.activation(sy[:], Yc[qq][:], AF.Square)
            yield lambda: D.tensor_tensor(sz[:], Zc[qq][:], Zc[qq][:], OP.mult)
            yield lambda: D.tensor_tensor(sx[:], sx[:], sy[:], OP.add)
            Ssum[qq] = p2p.tile([P, V, GQ], BF16, tag=f"s{qq}", name=f"s{qq}")
            yield lambda: G.tensor_tensor(Ssum[qq][:], sx[:], sz[:], OP.add)

        def zipper(gens):
            done = [False] * len(gens)
            while not all(done):
                for i, g in enumerate(gens):
                    if done[i]:
                        continue
                    try:
                        next(g)()
                    except StopIteration:
                        done[i] = True

        zipper([p2a_steps(qq) for qq in range(NQ)])

        # ---- eigen stage E-rest: [128, 32] fp32 analytic 3x3 eigensolve ----
        axx = small("axx"); ayy = small("ayy"); azz = small("azz")
        axy = small("axy"); axz = small("axz"); ayz = small("ayz")
        t0 = small("t0"); t1 = small("t1"); t2 = small("t2")
        t3 = small("t3"); t4 = small("t4"); t5 = small("t5")
        tt(D, t0, Sx, Sx, OP.mult)
        stt(D, axx, t0, -inv_s, Mxx, OP.mult, OP.add)
        tt(D, t1, Sy, Sy, OP.mult)
        stt(D, ayy, t1, -inv_s, Myy, OP.mult, OP.add)
        tt(D, t2, Sz, Sz, OP.mult)
        stt(D, azz, t2, -inv_s, Mzz, OP.mult, OP.add)
        tt(D, t3, Sx, Sy, OP.mult)
        stt(D, axy, t3, -inv_s, Mxy, OP.mult, OP.add)
        tt(D, t4, Sx, Sz, OP.mult)
        stt(D, axz, t4, -inv_s, Mxz, OP.mult, OP.add)
        tt(D, t5, Sy, Sz, OP.mult)
        stt(D, ayz, t5, -inv_s, Myz, OP.mult, OP.add)

        q = small("q")
        tt(D, t0, axx, ayy, OP.add)
        tt(D, t0, t0, azz, OP.add)
        ts(D, q, t0, 1.0 / 3.0)
        b11 = small("b11"); b22 = small("b22"); b33 = small("b33")
        tt(D, b11, axx, q, OP.subtract)
        tt(D, b22, ayy, q, OP.subtract)
        tt(D, b33, azz, q, OP.subtract)

        tt(G, t0, b11, b11, OP.mult)
        tt(G, t1, b22, b22, OP.mult)
        tt(G, t2, b33, b33, OP.mult)
        tt(G, t3, axy, axy, OP.mult)
        tt(G, t4, axz, axz, OP.mult)
        tt(G, t5, ayz, ayz, OP.mult)
        tt(D, t0, t0, t1, OP.add)
        tt(D, t0, t0, t2, OP.add)
        tt(G, t3, t3, t4, OP.add)
        tt(G, t3, t3, t5, OP.add)
        p2t = small("p2t")
        stt(D, p2t, t3, 2.0, t0, OP.mult, OP.add)
        p_ = small("p_")
        A.activation(p_[:], p2t[:], AF.Sqrt, scale=1.0 / 6.0)
        invp = small("invp")
        D.reciprocal(invp[:], p_[:])

        c11 = small("c11"); c22 = small("c22"); c33 = small("c33")
        c12 = small("c12"); c13 = small("c13"); c23 = small("c23")
        tt(D, c11, b11, invp, OP.mult)
        tt(D, c22, b22, invp, OP.mult)
        tt(D, c33, b33, invp, OP.mult)
        tt(G, c12, axy, invp, OP.mult)
        tt(G, c13, axz, invp, OP.mult)
        tt(G, c23, ayz, invp, OP.mult)

        # det(C)/2 -> r, clamped to [-1, 1]
        tt(D, t0, c22, c33, OP.mult)
        tt(G, t1, c23, c23, OP.mult)
        tt(D, t0, t0, t1, OP.subtract)
        tt(D, t0, t0, c11, OP.mult)
        tt(G, t2, c12, c33, OP.mult)
        tt(G, t3, c23, c13, OP.mult)
        tt(G, t2, t2, t3, OP.subtract)
        tt(G, t2, t2, c12, OP.mult)
        tt(D, t4, c12, c23, OP.mult)
        tt(D, t5, c22, c13, OP.mult)
        tt(D, t4, t4, t5, OP.subtract)
        tt(D, t4, t4, c13, OP.mult)
        tt(D, t0, t0, t2, OP.subtract)
        tt(D, t0, t0, t4, OP.add)
        r = small("r")
        ts(D, r, t0, 0.5, 1.0, OP.mult, OP.min)
        ts(D, r, r, -1.0, None, OP.max)

        # theta/4 = arctan(sqrt((1-r)/2) / (1 + sqrt((1+r)/2)))
        ts(D, t0, r, -0.5, 0.5, OP.mult, OP.add)
        ts(D, t1, r, 0.5, 0.5, OP.mult, OP.add)
        sa = small("sa"); sb = small("sb")
        A.activation(sa[:], t0[:], AF.Sqrt)
        A.activation(sb[:], t1[:], AF.Sqrt)
        ts(D, sb, sb, 1.0, None, OP.add)
        D.reciprocal(t2[:], sb[:])
        tt(D, t3, sa, t2, OP.mult)
        at4 = small("at4")
        A.activation(at4[:], t3[:], AF.Arctan)
        cmax = small("cmax"); smin = small("smin")
        A.activation(cmax[:], at4[:], AF.Sin, bias=bias_pi2[:, 0:1], scale=-4.0 / 3.0)
        A.activation(smin[:], at4[:], AF.Sin, bias=bias_pi6[:, 0:1], scale=4.0 / 3.0)

        w3 = small("w3"); w2 = small("w2")
        tt(D, t0, p_, cmax, OP.mult)
        stt(D, w3, t0, 2.0, q, OP.mult, OP.add)
        tt(G, t1, p_, smin, OP.mult)
        stt(D, t1, t1, -2.0, q, OP.mult, OP.add)      # w1
        stt(D, t2, q, 3.0, w3, OP.mult, OP.subtract)  # 3q - w3
        tt(D, w2, t2, t1, OP.subtract)
        invw3 = small("invw3")
        D.reciprocal(invw3[:], w3[:])
        dirwt = small("dirwt")
        tt(D, t0, w2, invw3, OP.mult)
        ts(D, dirwt, t0, -1.0, 1.0, OP.mult, OP.add)

        # B = A / w3 (9 entries, symmetric) written straight into feats
        tt(D, feats[:, :, 3], axx, invw3, OP.mult)
        tt(D, feats[:, :, 4], axy, invw3, OP.mult)
        A.copy(feats[:, :, 6], feats[:, :, 4])
        tt(D, feats[:, :, 5], axz, invw3, OP.mult)
        A.copy(feats[:, :, 9], feats[:, :, 5])
        tt(D, feats[:, :, 7], ayy, invw3, OP.mult)
        tt(D, feats[:, :, 8], ayz, invw3, OP.mult)
        A.copy(feats[:, :, 10], feats[:, :, 8])
        tt(D, feats[:, :, 11], azz, invw3, OP.mult)

        # principal eigenvector: cross products of rows of (A - w3 I)
        d1 = small("d1"); d2 = small("d2"); d3 = small("d3")
        tt(D, d1, axx, w3, OP.subtract)
        tt(D, d2, ayy, w3, OP.subtract)
        tt(D, d3, azz, w3, OP.subtract)
        u1 = small("u1"); u2 = small("u2"); u3 = small("u3")
        tt(G, t0, axy, ayz, OP.mult)
        tt(G, t1, d2, axz, OP.mult)
        tt(G, u1, t0, t1, OP.subtract)
        tt(D, t2, axy, axz, OP.mult)
        tt(D, t3, d1, ayz, OP.mult)
        tt(D, u2, t2, t3, OP.subtract)
        tt(G, t4, d1, d2, OP.mult)
        tt(G, t5, axy, axy, OP.mult)
        tt(G, u3, t4, t5, OP.subtract)
        k1 = small("k1"); k2 = small("k2")
        tt(D, t0, d2, d3, OP.mult)
        tt(D, t1, ayz, ayz, OP.mult)
        tt(D, k1, t0, t1, OP.subtract)
        tt(G, t2, ayz, axz, OP.mult)
        tt(G, t3, axy, d3, OP.mult)
        tt(G, k2, t2, t3, OP.subtract)
        # k3 = u1 (same formula)
        nu = small("nu"); nk = small("nk")
        nu1 = small("nu1")
        tt(D, nu1, u1, u1, OP.mult)
        tt(D, t0, u2, u2, OP.mult)
        tt(D, t1, u3, u3, OP.mult)
        tt(D, t0, t0, t1, OP.add)
        tt(D, nu, t0, nu1, OP.add)
        tt(G, t2, k1, k1, OP.mult)
        tt(G, t3, k2, k2, OP.mult)
        tt(G, t2, t2, t3, OP.add)
        tt(G, nk, t2, nu1, OP.add)
        m = small("m", U8)
        tt(D, m, nk, nu, OP.is_gt)
        e1 = small("e1"); e2 = small("e2"); e3 = small("e3"); ne = small("ne")
        D.select(e1[:], m[:], k1[:], u1[:])
        D.select(e2[:], m[:], k2[:], u2[:])
        D.select(e3[:], m[:], u1[:], u3[:])
        D.select(ne[:], m[:], nk[:], nu[:])
        rsn = small("rsn")
        A.activation(rsn[:], ne[:], AF.Sqrt)
        ts(D, rsn, rsn, 1e-30, None, OP.max)
        invn = small("invn")
        D.reciprocal(invn[:], rsn[:])
        v0x = small("v0x"); v0y = small("v0y"); v0z = small("v0z")
        tt(D, v0x, e1, invn, OP.mult)
        tt(D, v0y, e2, invn, OP.mult)
        tt(D, v0z, e3, invn, OP.mult)
        v0xb = small("v0xb", BF16); v0yb = small("v0yb", BF16)
        v0zb = small("v0zb", BF16)
        D.tensor_copy(v0xb[:], v0x[:])
        D.tensor_copy(v0yb[:], v0y[:])
        D.tensor_copy(v0zb[:], v0z[:])

        # ---- phase 2b: projections, residual norms, sign criterion ----
        sc = small("sc")
        G.memset(feats[:, :, 15], float(V))

        def p2b_steps(qq):
            a1 = p2p.tile([P, V, GQ], BF16, tag=f"a1{qq}", name=f"a1{qq}")
            a2 = p2p.tile([P, V, GQ], BF16, tag=f"a2{qq}", name=f"a2{qq}")
            a3 = p2p.tile([P, V, GQ], BF16, tag=f"a3{qq}", name=f"a3{qq}")
            yield lambda: D.tensor_tensor(a1[:], Xc[qq][:], bcq(v0xb, qq), OP.mult)
            yield lambda: G.tensor_tensor(a2[:], Yc[qq][:], bcq(v0yb, qq), OP.mult)
            yield lambda: G.tensor_tensor(a3[:], Zc[qq][:], bcq(v0zb, qq), OP.mult)
            x0 = p2p.tile([P, V, GQ], BF16, tag=f"x0{qq}", name=f"x0{qq}")
            yield lambda: D.tensor_tensor(x0[:], a1[:], a2[:], OP.add)
            yield lambda: D.tensor_tensor(x0[:], x0[:], a3[:], OP.add)
            q2 = p2p.tile([P, V, GQ], BF16, tag=f"q2{qq}", name=f"q2{qq}")
            yield lambda: A.activation(q2[:], x0[:], AF.Square)
            yield lambda: G.tensor_tensor(q2[:], Ssum[qq][:], q2[:], OP.subtract)
            yield lambda: ts(D, q2, q2, 0.0, None, OP.max)
            yield lambda: A.activation(q2[:], q2[:], AF.Sqrt)
            yield lambda: D.tensor_tensor(x0[:], x0[:], q2[:], OP.mult)
            yield lambda: D.tensor_reduce(sc[:, qq * GQ:(qq + 1) * GQ],
                                          x0[:].rearrange("p v g -> p g v"),
                                          axis=AX.X, op=OP.add)
            qs = slice(qq * GQ, (qq + 1) * GQ)
            yield lambda: ts(D, t0[:, qs], sc[:, qs], 0.0, -2.0, OP.is_lt, OP.mult)
            yield lambda: ts(D, t0[:, qs], t0[:, qs], 1.0, None, OP.add)
            yield lambda: tt(D, t1[:, qs], t0[:, qs], dirwt[:, qs], OP.mult)
            yield lambda: tt(D, feats[:, qs, 12], v0x[:, qs], t1[:, qs], OP.mult)
            yield lambda: tt(D, feats[:, qs, 13], v0y[:, qs], t1[:, qs], OP.mult)
            yield lambda: tt(D, feats[:, qs, 14], v0z[:, qs], t1[:, qs], OP.mult)
            yield lambda: nc.sync.dma_start(
                feats_d[qq * GQ:(qq + 1) * GQ].rearrange("g q f -> q g f"),
                feats[:, qs, :])

        zipper([p2b_steps(qq) for qq in range(NQ)])

    if not nc.is_finalized():
        nc.finalize()
    return nc


def kernel(data: np.ndarray, clusts: np.ndarray) -> np.ndarray:
    import ml_dtypes
    data = np.asarray(data, dtype=np.float32)
    clusts_np = np.asarray(clusts)
    C, S = clusts_np.shape
    assert (C, S) == (N_CLUSTS, CLUST_SIZE), (C, S)

    vox = data[:, 1:4]
    g3 = vox[clusts_np.reshape(-1).astype(np.int64)].reshape(C, S, 3)
    g3 = g3.astype(ml_dtypes.bfloat16)

    if "nc" not in _CACHED:
        _CACHED["nc"] = build_nc()
    nc = _CACHED["nc"]

    in_maps = []
    for c in range(N_CORES):
        a = g3[c * C_LOC:(c + 1) * C_LOC]          # [4096, 128, 3]
        vmt = np.ascontiguousarray(a.transpose(1, 0, 2))  # [128 vox, 4096, 3]
        # cluster-major seg-inner: [h, q, v, g] with c = (h*GH+g)*128 + q
        b = a.reshape(NH, GH, P, V, 3).transpose(0, 2, 3, 1, 4)
        b = np.ascontiguousarray(b)                # [2, 128, 128, 16, 3]
        in_maps.append({
            "xt": np.ascontiguousarray(vmt[:, :, 0]),
            "yt": np.ascontiguousarray(vmt[:, :, 1]),
            "zt": np.ascontiguousarray(vmt[:, :, 2]),
            "xc": np.ascontiguousarray(b[..., 0]),
            "yc": np.ascontiguousarray(b[..., 1]),
            "zc": np.ascontiguousarray(b[..., 2]),
        })

    res = run_bass_kernel_spmd(nc, in_maps, list(range(N_CORES)))
    out = np.concatenate(
        [res.results[c]["feats"].reshape(C_LOC, 16) for c in range(N_CORES)],
        axis=0)
    return out.astype(np.float32)
